# revision 86
# baseline (speedup 1.0000x reference)
"""Dense transformer block (B=4, T=2048, C=1024, H=16, FF=4096) on 8
Trainium2 NeuronCores.

Sharding: sequence-parallel, zero collectives. Core c handles batch
b = c // 2 and query-token half r = c % 2. The host permutes each
core's tokens so its OWN query tokens occupy local positions [0:1024)
(zigzag assignment: r=0 owns global [0:512)+[1536:2048), r=1 owns
[512:1536)), which removes the separate query-token layer-norm pass.
Each core redundantly computes LN1 + K/V for the full 2048-token
sequence of its batch, so no cross-core communication is needed.
Causality is enforced with per-core mask tensors (input data); the
attention chunk sets are uniform across cores: q-block 0 attends local
kv chunks {0..3, 8..11} (all maskable), q-block 1 attends all 16
chunks (slots 4..7 and 12..15 maskable).

All activations and weights are bf16 (fp32 PSUM accumulation); LN
gains are folded into the weights host-side and LN/QKV biases are
applied as per-partition bias in the PSUM->SBUF copies (V's bias via a
rank-1 accumulating matmul). Everything stays SBUF-resident between
phases - no DRAM bounce buffers.
"""
import numpy as np
import ml_dtypes

B, T, C = 4, 2048, 1024
H, D, FF = 16, 64, 4096
NC = 8
NKC = C // 128     # 8 feature chunks
NFFC = FF // 128   # 32
NVCH = T // 128    # 16 kv chunks
OWN = 1024         # own query tokens per core
EPS = 1e-5

CH_QB0 = [0, 1, 2, 3, 8, 9, 10, 11]          # qb0 chunk set (all masked)
CH_QB1 = list(range(16))                      # qb1 chunk set
QB1_MASKED = [4, 5, 6, 7, 12, 13, 14, 15]     # masked slots of qb1

_STATE = {}


def _build_program(with_vbias=True):
    import concourse.bacc as bacc
    import concourse.mybir as mybir
    from concourse.tile import TileContext

    F32 = mybir.dt.float32
    BF16 = mybir.dt.bfloat16
    AF = mybir.ActivationFunctionType
    OP = mybir.AluOpType

    nc = bacc.Bacc("TRN2", target_bir_lowering=False, debug=False,
                   num_devices=NC)

    xt_d = nc.dram_tensor("xt", [128, NKC, T], BF16, kind="ExternalInput")
    xqf_d = nc.dram_tensor("xqf", [128, NKC, OWN], F32, kind="ExternalInput")
    wq_d = nc.dram_tensor("wq", [8, 128, NKC, 128], BF16, kind="ExternalInput")
    wk_d = nc.dram_tensor("wk", [8, 128, NKC, 128], BF16, kind="ExternalInput")
    wv_d = nc.dram_tensor("wv", [2, 128, NKC, 512], BF16, kind="ExternalInput")
    wp_d = nc.dram_tensor("wp", [8, 128, NKC, 128], BF16, kind="ExternalInput")
    wf1_d = nc.dram_tensor("wf1", [NFFC, 128, NKC, 128], BF16,
                           kind="ExternalInput")
    wf2_d = nc.dram_tensor("wf2", [NKC, 128, NFFC, 128], BF16,
                           kind="ExternalInput")
    bq_d = nc.dram_tensor("bq", [128, 8], F32, kind="ExternalInput")
    bk_d = nc.dram_tensor("bk", [128, 8], F32, kind="ExternalInput")
    bv_d = nc.dram_tensor("bv", [1, 2, 512], BF16, kind="ExternalInput")
    bp_d = nc.dram_tensor("bp", [128, NKC], F32, kind="ExternalInput")
    bf1_d = nc.dram_tensor("bf1", [128, NFFC], F32, kind="ExternalInput")
    bf2_d = nc.dram_tensor("bf2", [128, NKC], F32, kind="ExternalInput")
    masks_d = nc.dram_tensor("masks", [128, 16, 512], BF16,
                             kind="ExternalInput")
    sel_d = nc.dram_tensor("sel", [16, 8, 128], BF16, kind="ExternalInput")
    ndg_d = nc.dram_tensor("ndg", [128, 128], BF16, kind="ExternalInput")
    ones4_d = nc.dram_tensor("ones4", [128, 4, 4], BF16, kind="ExternalInput")
    selln_d = nc.dram_tensor("selln", [4, 4, 128], BF16,
                             kind="ExternalInput")
    out_d = nc.dram_tensor("out", [128, NKC, OWN], F32, kind="ExternalOutput")

    def mm(ps, lhsT, rhs, start, stop):
        nc.tensor.matmul(ps, lhsT, rhs, start=start, stop=stop)

    with TileContext(nc, pool_alloc_mode="queue") as tc:
        consts_cm = tc.tile_pool(name="consts", bufs=1)
        consts = consts_cm.__enter__()

        ones128 = consts.tile([128, 1], BF16)
        nc.vector.memset(ones128, 1.0)
        onesrow = consts.tile([1, 128], BF16)
        nc.vector.memset(onesrow, 1.0)
        eps_t = consts.tile([4, 1], F32)
        nc.vector.memset(eps_t, EPS)
        # only the LN1-critical constants load up front; the rest are
        # emitted after LN1 so the first x-block DMA isn't queued
        # behind them on the sync engine
        ones4 = consts.tile([128, 4, 4], BF16)
        nc.sync.dma_start(out=ones4, in_=ones4_d[:, :, :])
        selln = consts.tile([4, 4, 128], BF16)
        nc.sync.dma_start(out=selln, in_=selln_d[:, :, :])
        bqt = consts.tile([128, 8], F32)
        bkt = consts.tile([128, 8], F32)
        bvt = consts.tile([1, 2, 512], BF16)
        bpt = consts.tile([128, NKC], F32)
        bf1t = consts.tile([128, NFFC], F32)
        bf2t = consts.tile([128, NKC], F32)
        selt = consts.tile([16, 8, 128], BF16)
        ndg = consts.tile([128, 128], BF16)

        def load_late_consts():
            nc.sync.dma_start(out=bqt, in_=bq_d[:, :])
            nc.sync.dma_start(out=bkt, in_=bk_d[:, :])
            nc.sync.dma_start(out=bvt, in_=bv_d[:, :, :])
            nc.sync.dma_start(out=bpt, in_=bp_d[:, :])
            nc.sync.dma_start(out=bf1t, in_=bf1_d[:, :])
            nc.sync.dma_start(out=bf2t, in_=bf2_d[:, :])
            nc.sync.dma_start(out=selt, in_=sel_d[:, :, :])
            nc.sync.dma_start(out=ndg, in_=ndg_d[:, :])

        # ---------------- layer norm over feature dim -----------------
        # dst = (src - mu) * rstd, with LN gain/bias folded into the
        # downstream weights/biases host-side. Stats accumulate via
        # ones-matmuls; squares on GpSimd; rstd via ACT Rsqrt.
        def ln_pass1(get_src, ntb, psum, work, wbig):
            # pass 1: per-block sums land in DISTINCT ROWS of one
            # [4,512] psum tile (selector lhsT columns), so the stats
            # chain runs ONCE, batched across blocks.
            ps_sums = psum.tile([4, 512], F32, tag="s")
            ps_sq = psum.tile([4, 512], F32, tag="q")
            sqs = [None] * ntb

            def sq_mms(tb):
                for k in range(NKC):
                    mm(ps_sq, ones4[:, tb, :], sqs[tb][:, k, :],
                       tb == 0 and k == 0, tb == ntb - 1 and k == NKC - 1)
                sqs[tb] = None

            for tb in range(ntb):
                src = get_src(tb)
                for k in range(NKC):
                    mm(ps_sums, ones4[:, tb, :], src[:, k, :],
                       tb == 0 and k == 0, tb == ntb - 1 and k == NKC - 1)
                sq = wbig.tile([128, NKC, 512], BF16, tag="sq")
                with nc.allow_low_precision(reason="bf16 x^2 for variance"):
                    nc.scalar.activation(out=sq, in_=src, func=AF.Square)
                sqs[tb] = sq
                if tb >= 2:
                    sq_mms(tb - 2)
            for tb in range(max(0, ntb - 2), ntb):
                sq_mms(tb)
            # batched stats across blocks (all tiles base partition 0)
            mut = work.tile([4, 512], F32, tag="mut")
            msqt = work.tile([4, 512], F32, tag="msqt")
            mu2 = work.tile([4, 512], F32, tag="mu2")
            rstdt = work.tile([4, 512], BF16, tag="rstdt")
            nmut = work.tile([4, 512], BF16, tag="nmut")
            nc.vector.tensor_scalar_mul(out=mut, in0=ps_sums,
                                        scalar1=1.0 / C)
            nc.vector.tensor_scalar_mul(out=msqt, in0=ps_sq,
                                        scalar1=1.0 / C)
            nc.vector.tensor_mul(out=mu2, in0=mut, in1=mut)
            nc.vector.tensor_sub(out=msqt, in0=msqt, in1=mu2)
            nc.scalar.activation(out=msqt, in_=msqt, func=AF.Sqrt,
                                 bias=eps_t, scale=1.0)
            with nc.allow_low_precision(reason="bf16 rstd"):
                nc.vector.reciprocal(out=rstdt, in_=msqt)
            with nc.allow_low_precision(reason="bf16 mu*rstd"):
                nc.vector.tensor_mul(out=nmut, in0=mut, in1=rstdt)
            return rstdt, nmut

        def ln_pass2(get_src, dst, ntb, psum, wbig, rstdt, nmut):
            # pass 2: broadcast via selector matmul + normalize in-place
            for tb in range(ntb):
                src = get_src(tb)
                sl = slice(tb * 512, (tb + 1) * 512)
                ps_bc = psum.tile([128, 2, 512], F32, tag="bc")
                mm(ps_bc[:, 0, :], selln[:, tb, :], rstdt, True, True)
                mm(ps_bc[:, 1, :], selln[:, tb, :], nmut, True, True)
                bc = wbig.tile([128, 2, 512], BF16, tag="bc")
                with nc.allow_low_precision(reason="bf16 LN broadcast"):
                    nc.vector.tensor_copy(out=bc, in_=ps_bc)
                with nc.allow_low_precision(reason="bf16 LN scale"):
                    nc.vector.tensor_mul(
                        out=dst[:, :, sl], in0=src,
                        in1=bc[:, 0:1, :].broadcast_to([128, NKC, 512]))
                with nc.allow_low_precision(reason="bf16 LN shift"):
                    nc.vector.tensor_sub(
                        out=dst[:, :, sl], in0=dst[:, :, sl],
                        in1=bc[:, 1:2, :].broadcast_to([128, NKC, 512]))

        def layer_norm(get_src, dst, ntb, psum, work, wbig):
            rstdt, nmut = ln_pass1(get_src, ntb, psum, work, wbig)
            ln_pass2(get_src, dst, ntb, psum, wbig, rstdt, nmut)

        # x2 (attention-block residual output) outlives kvp, so open it
        # first; kt/vt/q free before the MLP needs SBUF for m1t.
        x2_cm = tc.tile_pool(name="x2p", bufs=1)
        x2p = x2_cm.__enter__()
        x2t = x2p.tile([128, NKC, OWN], BF16)

        kv_cm = tc.tile_pool(name="kvp", bufs=1)
        kvp = kv_cm.__enter__()
        kt = kvp.tile([128, NKC, T], BF16)             # K, head-pair rows
        vt = kvp.tile([128, NVCH, H, 65], BF16)        # V + ones row
        q_sb = kvp.tile([128, 8, OWN], BF16)           # Q, head-pair rows
        nc.vector.memset(vt[:, :, :, 64:65], 1.0)

        # ======== Phase 1: LN1 over all 2048 tokens ========
        lnx_cm = tc.tile_pool(name="lnxp", bufs=1)
        lnxp = lnx_cm.__enter__()
        lnx = lnxp.tile([128, NKC, T], BF16)

        xt_cm = tc.tile_pool(name="xtp", bufs=2)
        xtp = xt_cm.__enter__()

        def ln1_src(tb):
            xtb = xtp.tile([128, NKC, 512], BF16, tag="xtb")
            nc.sync.dma_start(out=xtb,
                              in_=xt_d[:, :, tb * 512:(tb + 1) * 512])
            return xtb

        w1b_cm = tc.tile_pool(name="ln1wb", bufs=3)
        w1b = w1b_cm.__enter__()
        w1_cm = tc.tile_pool(name="ln1w", bufs=2)
        w1 = w1_cm.__enter__()
        ps1_cm = tc.tile_pool(name="ln1ps", bufs=2, space="PSUM")
        ps1 = ps1_cm.__enter__()
        # two half-calls, pass1s first: the second half's sum-matmuls
        # keep the PE fed while the first half's stats chain runs
        src_b = lambda tb: ln1_src(tb + 2)
        ra = ln_pass1(ln1_src, 2, ps1, w1, w1b)
        load_late_consts()
        rb = ln_pass1(src_b, 2, ps1, w1, w1b)
        ln_pass2(ln1_src, lnx, 2, ps1, w1b, *ra)
        ln_pass2(src_b, lnx[:, :, 1024:2048], 2, ps1, w1b, *rb)
        ps1_cm.__exit__(None, None, None)
        w1_cm.__exit__(None, None, None)
        w1b_cm.__exit__(None, None, None)
        xt_cm.__exit__(None, None, None)

        # ======== Phase 2: QKV projections ========
        qw_cm = tc.tile_pool(name="qkvw", bufs=3)
        qw = qw_cm.__enter__()
        psq_cm = tc.tile_pool(name="qkvps", bufs=6, space="PSUM")
        psq = psq_cm.__enter__()

        # Q (own tokens = local [0:1024)), scaled by 1/sqrt(D)
        for oc in range(8):
            wt = qw.tile([128, NKC, 128], BF16, tag="w")
            nc.sync.dma_start(out=wt, in_=wq_d[oc])
            for tb in range(2):
                sl = slice(tb * 512, (tb + 1) * 512)
                ps = psq.tile([128, 512], F32, tag="mm")
                for k in range(NKC):
                    mm(ps, wt[:, k, :], lnx[:, k, sl], k == 0, k == NKC - 1)
                with nc.allow_low_precision(reason="bf16 q"):
                    nc.vector.tensor_scalar(
                        out=q_sb[:, oc, sl], in0=ps,
                        scalar1=bqt[:, oc:oc + 1], scalar2=1.0 / np.sqrt(D),
                        op0=OP.add, op1=OP.mult)
        # K (all tokens)
        for oc in range(8):
            wt = qw.tile([128, NKC, 128], BF16, tag="w")
            nc.sync.dma_start(out=wt, in_=wk_d[oc])
            for tb in range(4):
                sl = slice(tb * 512, (tb + 1) * 512)
                ps = psq.tile([128, 512], F32, tag="mm")
                for k in range(NKC):
                    mm(ps, wt[:, k, :], lnx[:, k, sl], k == 0, k == NKC - 1)
                with nc.allow_low_precision(reason="bf16 k"):
                    nc.vector.tensor_scalar(
                        out=kt[:, oc, sl], in0=ps,
                        scalar1=bkt[:, oc:oc + 1], scalar2=1.0,
                        op0=OP.add, op1=OP.mult)
        # V (all tokens, token-major), bias via rank-1 matmul
        for g in range(2):
            wv = qw.tile([128, NKC, 512], BF16, tag="wv")
            nc.sync.dma_start(out=wv, in_=wv_d[g])
            for cch in range(NVCH):
                ps = psq.tile([128, 512], F32, tag="mm")
                for k in range(NKC):
                    mm(ps, lnx[:, k, cch * 128:(cch + 1) * 128], wv[:, k, :],
                       k == 0, k == NKC - 1 and not with_vbias)
                if with_vbias:
                    mm(ps, onesrow, bvt[0:1, g, :], False, True)
                with nc.allow_low_precision(reason="bf16 v"):
                    nc.vector.tensor_copy(
                        out=vt[:, cch, g * 8:(g + 1) * 8, 0:64], in_=ps)

        psq_cm.__exit__(None, None, None)
        qw_cm.__exit__(None, None, None)
        lnx_cm.__exit__(None, None, None)

        # ======== Phase 3: attention ========
        y_cm = tc.tile_pool(name="yp", bufs=1)
        yp = y_cm.__enter__()
        y_sb = yp.tile([128, NKC, OWN], BF16)
        mt = yp.tile([128, 16, 512], BF16)             # causal masks
        nc.sync.dma_start(out=mt, in_=masks_d[:, :, :])
        xqf_t = yp.tile([128, NKC, OWN], F32)
        for oc in range(8):
            nc.sync.dma_start(out=xqf_t[:, oc, :], in_=xqf_d[:, oc, :])

        prw_cm = tc.tile_pool(name="prw", bufs=3)
        prw = prw_cm.__enter__()
        attw_cm = tc.tile_pool(name="attw", bufs=4)
        attw = attw_cm.__enter__()
        attsm_cm = tc.tile_pool(name="attsm", bufs=1)
        attsm = attsm_cm.__enter__()
        pss_cm = tc.tile_pool(name="attps", bufs=3, space="PSUM")
        pss = pss_cm.__enter__()
        psy_cm = tc.tile_pool(name="attpy", bufs=1, space="PSUM")
        psy = psy_cm.__enter__()

        # proj is emitted interleaved with attention so the qb1
        # softmax-tail latency hides under proj matmuls; psum comes
        # from the scores pool.
        def proj_tb(tb):
            sl = slice(tb * 512, (tb + 1) * 512)
            for oc in range(8):
                wt = prw.tile([128, NKC, 128], BF16, tag="w")
                nc.sync.dma_start(out=wt, in_=wp_d[oc])
                ps = pss.tile([128, 2, 512], F32, tag="s")
                for k in range(NKC):
                    mm(ps[:, 0, :], wt[:, k, :], y_sb[:, k, sl],
                       k == 0, k == NKC - 1)
                with nc.allow_low_precision(reason="bf16 x2"):
                    nc.vector.scalar_tensor_tensor(
                        out=x2t[:, oc, sl], in0=ps[:, 0, :],
                        scalar=bpt[:, oc:oc + 1],
                        in1=xqf_t[:, oc, sl], op0=OP.add, op1=OP.add)

        # y_sb first holds UNNORMALIZED head outputs. Softmax
        # denominators collect on partition 0 (engine partition-base
        # rules), one SBUF->SBUF DMA scatters them across 16
        # partitions, then a single batched reciprocal + selector
        # matmul broadcasts normalize y. qb0's tail hides under qb1.
        for qb in range(2):
            chunks = CH_QB0 if qb == 0 else CH_QB1
            qsl = slice(qb * 512, (qb + 1) * 512)
            nch = len(chunks)
            dtmp = attsm.tile([1, 16, 512], BF16, tag="dtmp")
            for hp in range(8):
                ha, hb = 2 * hp, 2 * hp + 1
                ps_ya = psy.tile([65, 512], F32, tag="ya")
                ps_yb = psy.tile([65, 512], F32, tag="yb")
                # software pipeline: scores run one chunk ahead of attV
                ps_list = [None] * nch
                ptm_list = [None] * nch

                def scores(i):
                    ci = chunks[i]
                    csl = slice(ci * 128, (ci + 1) * 128)
                    if qb == 0:
                        slot = i
                    else:
                        slot = 8 + QB1_MASKED.index(ci) if ci in QB1_MASKED \
                            else None
                    ps_s = pss.tile([128, 2, 512], F32, tag="s")
                    for j, psl in ((0, slice(0, 64)), (1, slice(64, 128))):
                        mm(ps_s[:, j, :], kt[psl, hp, csl], q_sb[psl, hp, qsl],
                           True, slot is None)
                    if slot is not None:
                        # causal mask as -60000 * maskc accumulated
                        # into the scores via a diagonal matmul
                        for j in range(2):
                            mm(ps_s[:, j, :], ndg, mt[:, slot, :],
                               False, True)
                    ps_list[i] = ps_s

                def softmax(i):
                    pt = attw.tile([128, 2, 512], BF16, tag="pt")
                    with nc.allow_low_precision(reason="bf16 softmax"):
                        nc.scalar.activation(out=pt, in_=ps_list[i],
                                             func=AF.Exp)
                    ptm_list[i] = pt

                def attv(i):
                    ci = chunks[i]
                    ptm = ptm_list[i]
                    mm(ps_ya, vt[:, ci, ha, :], ptm[:, 0, :],
                       i == 0, i == nch - 1)
                    mm(ps_yb, vt[:, ci, hb, :], ptm[:, 1, :],
                       i == 0, i == nch - 1)
                    ps_list[i] = None
                    ptm_list[i] = None

                scores(0)
                softmax(0)
                scores(1)
                softmax(1)
                for i in range(2, nch):
                    scores(i)
                    softmax(i)
                    attv(i - 2)
                attv(nch - 2)
                attv(nch - 1)

                for h, ps_y in ((ha, ps_ya), (hb, ps_yb)):
                    with nc.allow_low_precision(reason="bf16 y raw"):
                        nc.vector.tensor_copy(
                            out=y_sb[(h % 2) * 64:(h % 2) * 64 + 64,
                                     h // 2, qsl],
                            in_=ps_y[0:64, :])
                    hi = 2 * hp + h % 2
                    with nc.allow_low_precision(reason="bf16 denom"):
                        nc.vector.tensor_copy(out=dtmp[0:1, hi, :],
                                              in_=ps_y[64:65, :])

            # per-qb normalize tail: selector-matmul broadcasts both
            # heads' reciprocal rows into one [128, 512] tile per pair
            den16 = attsm.tile([16, 512], BF16, tag="den16")
            nc.sync.dma_start(out=den16, in_=dtmp[0:1, :, :])
            rden = attsm.tile([16, 512], BF16, tag="rden")
            with nc.allow_low_precision(reason="bf16 softmax denom"):
                nc.vector.reciprocal(out=rden, in_=den16)
            if qb == 1:
                # keep the PE fed while the qb1 denominator DMA +
                # reciprocal complete
                proj_tb(0)
            for hp in range(8):
                rb = pss.tile([128, 2, 512], F32, tag="s")
                mm(rb[:, 0, :], selt[:, hp, :], rden, True, True)
                ysl = y_sb[:, hp, qsl]
                with nc.allow_low_precision(reason="bf16 y norm"):
                    nc.vector.tensor_mul(out=ysl, in0=ysl, in1=rb[:, 0, :])
        proj_tb(1)

        psy_cm.__exit__(None, None, None)
        pss_cm.__exit__(None, None, None)
        attsm_cm.__exit__(None, None, None)
        attw_cm.__exit__(None, None, None)
        prw_cm.__exit__(None, None, None)
        y_cm.__exit__(None, None, None)
        kv_cm.__exit__(None, None, None)

        # ======== Phase 5: LN2 ========
        ln2x_cm = tc.tile_pool(name="ln2xp", bufs=1)
        ln2xp = ln2x_cm.__enter__()
        ln2x = ln2xp.tile([128, NKC, OWN], BF16)
        w2b_cm = tc.tile_pool(name="ln2wb", bufs=2)
        w2b = w2b_cm.__enter__()
        w2_cm = tc.tile_pool(name="ln2w", bufs=1)
        w2 = w2_cm.__enter__()
        ps2_cm = tc.tile_pool(name="ln2ps", bufs=2, space="PSUM")
        ps2 = ps2_cm.__enter__()
        layer_norm(lambda tb: x2t[:, :, tb * 512:(tb + 1) * 512],
                   ln2x, OWN // 512, ps2, w2, w2b)
        ps2_cm.__exit__(None, None, None)
        w2_cm.__exit__(None, None, None)
        w2b_cm.__exit__(None, None, None)

        # ======== Phase 6: MLP ========
        mlp_cm = tc.tile_pool(name="mlpp", bufs=1)
        mlpp = mlp_cm.__enter__()
        m1t = mlpp.tile([128, NFFC, OWN], BF16)
        mw1_cm = tc.tile_pool(name="mw1", bufs=3)
        mw1 = mw1_cm.__enter__()
        mw2_cm = tc.tile_pool(name="mw2", bufs=2)
        mw2 = mw2_cm.__enter__()
        mo_cm = tc.tile_pool(name="mo", bufs=3)
        mo = mo_cm.__enter__()
        psm_cm = tc.tile_pool(name="mlpps", bufs=4, space="PSUM")
        psm = psm_cm.__enter__()

        # tb-outer: all of block 0's fc1 runs before anything needs
        # ln2x block 1, hiding the LN2 stats tail (wf1 loads twice;
        # the extra 8MB of DMA hides under 55us of matmuls)
        for tb in range(2):
            sl = slice(tb * 512, (tb + 1) * 512)
            for ffc in range(NFFC):
                wt = mw1.tile([128, NKC, 128], BF16, tag="w1")
                nc.sync.dma_start(out=wt, in_=wf1_d[ffc])
                ps = psm.tile([128, 512], F32, tag="mm1")
                for k in range(NKC):
                    mm(ps, wt[:, k, :], ln2x[:, k, sl], k == 0, k == NKC - 1)
                with nc.allow_low_precision(reason="bf16 mlp hidden"):
                    nc.vector.tensor_scalar(
                        out=m1t[:, ffc, sl], in0=ps,
                        scalar1=bf1t[:, ffc:ffc + 1], scalar2=0.0,
                        op0=OP.add, op1=OP.max)
        for oc in range(NKC):
            wt2 = mw2.tile([128, NFFC, 128], BF16, tag="w2")
            nc.sync.dma_start(out=wt2, in_=wf2_d[oc])
            for tb in range(2):
                sl = slice(tb * 512, (tb + 1) * 512)
                ps = psm.tile([128, 512], F32, tag="mm2")
                for k in range(NFFC):
                    mm(ps, wt2[:, k, :], m1t[:, k, sl], k == 0, k == NFFC - 1)
                ot = mo.tile([128, 512], F32, tag="ot")
                nc.vector.scalar_tensor_tensor(
                    out=ot, in0=ps, scalar=bf2t[:, oc:oc + 1],
                    in1=x2t[:, oc, sl], op0=OP.add, op1=OP.add)
                nc.sync.dma_start(out=out_d[:, oc, sl], in_=ot)

        psm_cm.__exit__(None, None, None)
        mo_cm.__exit__(None, None, None)
        mw2_cm.__exit__(None, None, None)
        mw1_cm.__exit__(None, None, None)
        mlp_cm.__exit__(None, None, None)
        ln2x_cm.__exit__(None, None, None)
        x2_cm.__exit__(None, None, None)
        consts_cm.__exit__(None, None, None)

    nc.compile()
    return nc


class _SpmdRunner:
    def __init__(self, nc, n_cores=NC):
        import jax
        from jax.sharding import Mesh, PartitionSpec
        from jax.experimental.shard_map import shard_map
        import concourse.mybir as mybir
        from concourse import bass2jax
        bass2jax.install_neuronx_cc_hook()
        self.jax = jax
        self.n_cores = n_cores
        partition_name = (
            nc.partition_id_tensor.name if nc.partition_id_tensor else None)
        in_names, out_names, out_avals = [], [], []
        for alloc in nc.m.functions[0].allocations:
            if not isinstance(alloc, mybir.MemoryLocationSet):
                continue
            name = alloc.memorylocations[0].name
            if alloc.kind == "ExternalInput":
                if name != partition_name:
                    in_names.append(name)
            elif alloc.kind == "ExternalOutput":
                out_names.append(name)
                out_avals.append(jax.core.ShapedArray(
                    tuple(alloc.tensor_shape), mybir.dt.np(alloc.dtype)))
        self.in_names = in_names
        self.out_names = out_names
        self.out_avals = out_avals
        all_in = in_names + out_names
        if partition_name is not None:
            all_in.append(partition_name)

        def _body(*args):
            operands = list(args)
            if partition_name is not None:
                operands.append(bass2jax.partition_id_tensor())
            outs = bass2jax._bass_exec_p.bind(
                *operands, out_avals=tuple(out_avals),
                in_names=tuple(all_in), out_names=tuple(out_names),
                lowering_input_output_aliases=(),
                sim_require_finite=True, sim_require_nnan=True, nc=nc)
            return tuple(outs)

        devices = jax.devices()[:n_cores]
        self.mesh = Mesh(np.asarray(devices), ("core",))
        n_io = len(in_names) + len(out_names)
        self.fn = jax.jit(
            shard_map(_body, mesh=self.mesh,
                      in_specs=(PartitionSpec("core"),) * n_io,
                      out_specs=(PartitionSpec("core"),) * len(out_names),
                      check_rep=False),
            keep_unused=True)
        self._dev_in = None

    def put_inputs(self, in_maps):
        from jax.sharding import NamedSharding, PartitionSpec
        jax = self.jax
        sh = NamedSharding(self.mesh, PartitionSpec("core"))
        concat = []
        for name in self.in_names:
            arrs = [np.asarray(in_maps[c][name]) for c in range(self.n_cores)]
            concat.append(jax.device_put(np.concatenate(arrs, axis=0), sh))
        for av in self.out_avals:
            z = np.zeros((self.n_cores * av.shape[0], *av.shape[1:]), av.dtype)
            concat.append(jax.device_put(z, sh))
        self._dev_in = concat

    def run(self):
        jax = self.jax
        outs = self.fn(*self._dev_in)
        jax.block_until_ready(outs)
        results = []
        for c in range(self.n_cores):
            d = {}
            for i, name in enumerate(self.out_names):
                av = self.out_avals[i]
                d[name] = np.asarray(outs[i]).reshape(
                    self.n_cores, *av.shape)[c]
            results.append(d)
        return results

    def time_exec(self, warmup=3, m1=4, m2=12, reps=3, trials=6):
        """Estimate per-call device time by dispatching bursts of m1 and
        m2 back-to-back calls and differencing, which cancels the
        constant dispatch/RTT overhead of the axon tunnel. Dispatch
        stalls only ever inflate a burst, so the minimum over several
        trials is the tightest estimate of true device throughput."""
        import time
        jax = self.jax
        for _ in range(warmup):
            jax.block_until_ready(self.fn(*self._dev_in))

        def burst(m):
            t0 = time.perf_counter()
            outs = None
            for _ in range(m):
                outs = self.fn(*self._dev_in)
            jax.block_until_ready(outs)
            return time.perf_counter() - t0

        t1s, t2s = [], []
        for _ in range(trials):
            for _ in range(reps):
                t1s.append(burst(m1))
                t2s.append(burst(m2))
        est = (min(t2s) - min(t1s)) / (m2 - m1)
        if est <= 0:
            # dispatch noise overwhelmed the diff; fall back to the
            # tightest whole-burst bound (includes per-call overhead)
            est = min(min(t2s) / m2, min(t1s) / m1)
        return est


def _get_runner(with_vbias=None):
    if with_vbias is None:
        if "last" in _STATE:
            return _STATE["last"]
        with_vbias = True
    key = ("runner", with_vbias)
    if key not in _STATE:
        nc = _build_program(with_vbias)
        _STATE[key] = _SpmdRunner(nc)
    _STATE["last"] = _STATE[key]
    return _STATE[key]


def _perm(r):
    """Per-core token permutation: own query tokens first (zigzag)."""
    if r == 0:
        return np.concatenate([np.arange(0, 512), np.arange(1536, 2048),
                               np.arange(512, 1536)])
    return np.concatenate([np.arange(512, 1536), np.arange(0, 512),
                           np.arange(1536, 2048)])


def _prep_in_maps(x, W_attn, W_proj, b_proj, W_fc1, b_fc1, W_fc2, b_fc2,
                  ln1_g, ln1_b, ln2_g, ln2_b):
    f32 = np.float32
    bf16 = ml_dtypes.bfloat16
    x = np.asarray(x, f32)
    W_attn = np.asarray(W_attn, f32)
    g1 = np.asarray(ln1_g, f32)
    b1 = np.asarray(ln1_b, f32)
    g2 = np.asarray(ln2_g, f32)
    b2 = np.asarray(ln2_b, f32)
    Wq = g1[:, None] * W_attn[:, 0:C]
    Wk = g1[:, None] * W_attn[:, C:2 * C]
    Wv = g1[:, None] * W_attn[:, 2 * C:3 * C]
    bq = b1 @ W_attn[:, 0:C]
    bk = b1 @ W_attn[:, C:2 * C]
    bv = b1 @ W_attn[:, 2 * C:3 * C]
    Wfc1 = g2[:, None] * np.asarray(W_fc1, f32)
    bfc1 = np.asarray(b_fc1, f32) + b2 @ np.asarray(W_fc1, f32)

    def lhs_tiles(W, nout):
        # [C, nout*128] -> [nout, 128p, NKC, 128m]
        return np.ascontiguousarray(
            W.reshape(NKC, 128, nout, 128).transpose(2, 1, 0, 3)).astype(bf16)

    wq = lhs_tiles(Wq, 8)
    wk = lhs_tiles(Wk, 8)
    wv = np.ascontiguousarray(
        Wv.reshape(NKC, 128, 2, 512).transpose(2, 1, 0, 3)).astype(bf16)
    wp = lhs_tiles(np.asarray(W_proj, f32), 8)
    wf1 = lhs_tiles(Wfc1, NFFC)
    wf2 = np.ascontiguousarray(
        np.asarray(W_fc2, f32).reshape(NFFC, 128, NKC, 128)
        .transpose(2, 1, 0, 3)).astype(bf16)

    def vec(v, nk):
        return np.ascontiguousarray(np.asarray(v, f32).reshape(nk, 128).T)

    sel = np.zeros((16, 8, 128), f32)
    for hp in range(8):
        sel[2 * hp, hp, 0:64] = 1.0
        sel[2 * hp + 1, hp, 64:128] = 1.0
    # LN selectors: ones4 routes block tb's sum to psum row tb; selln
    # broadcasts rnt row tb (rstd, slot 2tb) / row 32+tb (mu*rstd,
    # slot 2tb+1) across all 128 partitions
    ones4 = np.zeros((128, 4, 4), f32)
    for tb in range(4):
        ones4[:, tb, tb] = 1.0
    selln = np.zeros((4, 4, 128), f32)
    for tb in range(4):
        selln[tb, tb, :] = 1.0

    shared = {
        "wq": wq, "wk": wk, "wv": wv, "wp": wp, "wf1": wf1, "wf2": wf2,
        "sel": sel.astype(bf16),
        "ndg": (-60000.0 * np.eye(128, dtype=f32)).astype(bf16),
        "ones4": ones4.astype(bf16),
        "selln": selln.astype(bf16),
        "bq": vec(bq, 8), "bk": vec(bk, 8),
        "bv": np.ascontiguousarray(bv.reshape(1, 2, 512)).astype(bf16),
        "bp": vec(np.asarray(b_proj, f32), NKC),
        "bf1": vec(bfc1, NFFC),
        "bf2": vec(np.asarray(b_fc2, f32), NKC),
    }

    in_maps = []
    for c in range(NC):
        b, r = c // 2, c % 2
        perm = _perm(r)
        xp = x[b][perm]                       # [T, C] local token order
        xt = np.ascontiguousarray(
            xp.T.reshape(NKC, 128, T).transpose(1, 0, 2)).astype(bf16)
        xqf = np.ascontiguousarray(
            xp[:OWN].T.reshape(NKC, 128, OWN).transpose(1, 0, 2))
        # masks[p, slot, qi]: slots 0..7 = qb0 chunks CH_QB0;
        # slots 8..15 = qb1 chunks QB1_MASKED. 1 where kv_g <= q_g.
        # complement masks: 1 where attention is FORBIDDEN (kv > q)
        m = np.zeros((128, 16, 512), f32)
        for j, ci in enumerate(CH_QB0):
            gkv = perm[ci * 128:(ci + 1) * 128]
            gq = perm[0:512]
            m[:, j, :] = (gkv[:, None] > gq[None, :]).astype(f32)
        for j, ci in enumerate(QB1_MASKED):
            gkv = perm[ci * 128:(ci + 1) * 128]
            gq = perm[512:1024]
            m[:, 8 + j, :] = (gkv[:, None] > gq[None, :]).astype(f32)
        d = {"xt": xt, "xqf": xqf, "masks": m.astype(bf16)}
        d.update(shared)
        in_maps.append(d)
    return in_maps


def kernel(x, W_attn, W_proj, b_proj, W_fc1, b_fc1, W_fc2, b_fc2,
           ln1_g, ln1_b, ln2_g, ln2_b):
    bv = np.asarray(ln1_b, np.float32) @ np.asarray(
        W_attn, np.float32)[:, 2 * C:3 * C]
    runner = _get_runner(bool(np.any(bv != 0.0)))
    in_maps = _prep_in_maps(x, W_attn, W_proj, b_proj, W_fc1, b_fc1,
                            W_fc2, b_fc2, ln1_g, ln1_b, ln2_g, ln2_b)
    runner.put_inputs(in_maps)
    results = runner.run()
    out = np.empty((B, T, C), np.float32)
    for c in range(NC):
        b, r = c // 2, c % 2
        ot = results[c]["out"]                # [128, NKC, OWN]
        feat = ot.transpose(1, 0, 2).reshape(C, OWN)
        out[b, _perm(r)[:OWN], :] = feat.T
    return out


# revision 90
# speedup vs baseline: 3.4250x; 3.4250x over previous
"""Dense transformer block (B=4, T=2048, C=1024, H=16, FF=4096) on 8
Trainium2 NeuronCores.

Sharding: sequence-parallel, zero collectives. Core c handles batch
b = c // 2 and query-token half r = c % 2. The host permutes each
core's tokens so its OWN query tokens occupy local positions [0:1024)
(zigzag assignment: r=0 owns global [0:512)+[1536:2048), r=1 owns
[512:1536)), which removes the separate query-token layer-norm pass.
Each core redundantly computes LN1 + K/V for the full 2048-token
sequence of its batch, so no cross-core communication is needed.
Causality is enforced with per-core mask tensors (input data); the
attention chunk sets are uniform across cores: q-block 0 attends local
kv chunks {0..3, 8..11} (all maskable), q-block 1 attends all 16
chunks (slots 4..7 and 12..15 maskable).

All activations and weights are bf16 (fp32 PSUM accumulation); LN
gains are folded into the weights host-side and LN/QKV biases are
applied as per-partition bias in the PSUM->SBUF copies (V's bias via a
rank-1 accumulating matmul). Everything stays SBUF-resident between
phases - no DRAM bounce buffers.
"""
import numpy as np
import ml_dtypes

B, T, C = 4, 2048, 1024
H, D, FF = 16, 64, 4096
NC = 8
NKC = C // 128     # 8 feature chunks
NFFC = FF // 128   # 32
NVCH = T // 128    # 16 kv chunks
OWN = 1024         # own query tokens per core
EPS = 1e-5

CH_QB0 = [0, 1, 2, 3, 8, 9, 10, 11]          # qb0 chunk set (all masked)
CH_QB1 = list(range(16))                      # qb1 chunk set
QB1_MASKED = [4, 5, 6, 7, 12, 13, 14, 15]     # masked slots of qb1

_STATE = {}


def _build_program(with_vbias=True):
    import concourse.bacc as bacc
    import concourse.mybir as mybir
    from concourse.tile import TileContext

    F32 = mybir.dt.float32
    BF16 = mybir.dt.bfloat16
    AF = mybir.ActivationFunctionType
    OP = mybir.AluOpType

    nc = bacc.Bacc("TRN2", target_bir_lowering=False, debug=False,
                   num_devices=NC)

    xt_d = nc.dram_tensor("xt", [128, NKC, T], BF16, kind="ExternalInput")
    xqf_d = nc.dram_tensor("xqf", [128, NKC, OWN], F32, kind="ExternalInput")
    wq_d = nc.dram_tensor("wq", [8, 128, NKC, 128], BF16, kind="ExternalInput")
    wk_d = nc.dram_tensor("wk", [8, 128, NKC, 128], BF16, kind="ExternalInput")
    wv_d = nc.dram_tensor("wv", [2, 128, NKC, 512], BF16, kind="ExternalInput")
    wp_d = nc.dram_tensor("wp", [8, 128, NKC, 128], BF16, kind="ExternalInput")
    wf1_d = nc.dram_tensor("wf1", [NFFC, 128, NKC, 128], BF16,
                           kind="ExternalInput")
    wf2_d = nc.dram_tensor("wf2", [NKC, 128, NFFC, 128], BF16,
                           kind="ExternalInput")
    bq_d = nc.dram_tensor("bq", [128, 8], F32, kind="ExternalInput")
    bk_d = nc.dram_tensor("bk", [128, 8], F32, kind="ExternalInput")
    bv_d = nc.dram_tensor("bv", [1, 2, 512], BF16, kind="ExternalInput")
    bp_d = nc.dram_tensor("bp", [128, NKC], F32, kind="ExternalInput")
    bf1_d = nc.dram_tensor("bf1", [128, NFFC], F32, kind="ExternalInput")
    bf2_d = nc.dram_tensor("bf2", [128, NKC], F32, kind="ExternalInput")
    masks_d = nc.dram_tensor("masks", [128, 16, 512], BF16,
                             kind="ExternalInput")
    sel_d = nc.dram_tensor("sel", [16, 8, 128], BF16, kind="ExternalInput")
    ndg_d = nc.dram_tensor("ndg", [128, 128], BF16, kind="ExternalInput")
    ones4_d = nc.dram_tensor("ones4", [128, 4, 4], BF16, kind="ExternalInput")
    selln_d = nc.dram_tensor("selln", [4, 4, 128], BF16,
                             kind="ExternalInput")
    out_d = nc.dram_tensor("out", [128, NKC, OWN], F32, kind="ExternalOutput")

    def mm(ps, lhsT, rhs, start, stop):
        nc.tensor.matmul(ps, lhsT, rhs, start=start, stop=stop)

    with TileContext(nc, pool_alloc_mode="queue") as tc:
        consts_cm = tc.tile_pool(name="consts", bufs=1)
        consts = consts_cm.__enter__()

        ones128 = consts.tile([128, 1], BF16)
        nc.vector.memset(ones128, 1.0)
        onesrow = consts.tile([1, 128], BF16)
        nc.vector.memset(onesrow, 1.0)
        eps_t = consts.tile([4, 1], F32)
        nc.vector.memset(eps_t, EPS)
        # only the LN1-critical constants load up front; the rest are
        # emitted after LN1 so the first x-block DMA isn't queued
        # behind them on the sync engine
        ones4 = consts.tile([128, 4, 4], BF16)
        nc.sync.dma_start(out=ones4, in_=ones4_d[:, :, :])
        selln = consts.tile([4, 4, 128], BF16)
        nc.sync.dma_start(out=selln, in_=selln_d[:, :, :])
        bqt = consts.tile([128, 8], F32)
        bkt = consts.tile([128, 8], F32)
        bvt = consts.tile([1, 2, 512], BF16)
        bpt = consts.tile([128, NKC], F32)
        bf1t = consts.tile([128, NFFC], F32)
        bf2t = consts.tile([128, NKC], F32)
        selt = consts.tile([16, 8, 128], BF16)
        ndg = consts.tile([128, 128], BF16)

        def load_late_consts():
            nc.sync.dma_start(out=bqt, in_=bq_d[:, :])
            nc.sync.dma_start(out=bkt, in_=bk_d[:, :])
            nc.sync.dma_start(out=bvt, in_=bv_d[:, :, :])
            nc.sync.dma_start(out=bpt, in_=bp_d[:, :])
            nc.sync.dma_start(out=bf1t, in_=bf1_d[:, :])
            nc.sync.dma_start(out=bf2t, in_=bf2_d[:, :])
            nc.sync.dma_start(out=selt, in_=sel_d[:, :, :])
            nc.sync.dma_start(out=ndg, in_=ndg_d[:, :])

        # ---------------- layer norm over feature dim -----------------
        # dst = (src - mu) * rstd, with LN gain/bias folded into the
        # downstream weights/biases host-side. Stats accumulate via
        # ones-matmuls; squares on GpSimd; rstd via ACT Rsqrt.
        def ln_pass1(get_src, ntb, psum, work, wbig):
            # pass 1: per-block sums land in DISTINCT ROWS of one
            # [4,512] psum tile (selector lhsT columns), so the stats
            # chain runs ONCE, batched across blocks.
            ps_sums = psum.tile([4, 512], F32, tag="s")
            ps_sq = psum.tile([4, 512], F32, tag="q")
            sqs = [None] * ntb

            def sq_mms(tb):
                for k in range(NKC):
                    mm(ps_sq, ones4[:, tb, :], sqs[tb][:, k, :],
                       tb == 0 and k == 0, tb == ntb - 1 and k == NKC - 1)
                sqs[tb] = None

            for tb in range(ntb):
                src = get_src(tb)
                for k in range(NKC):
                    mm(ps_sums, ones4[:, tb, :], src[:, k, :],
                       tb == 0 and k == 0, tb == ntb - 1 and k == NKC - 1)
                sq = wbig.tile([128, NKC, 512], BF16, tag="sq")
                with nc.allow_low_precision(reason="bf16 x^2 for variance"):
                    nc.scalar.activation(out=sq, in_=src, func=AF.Square)
                sqs[tb] = sq
                if tb >= 2:
                    sq_mms(tb - 2)
            for tb in range(max(0, ntb - 2), ntb):
                sq_mms(tb)
            # batched stats across blocks (all tiles base partition 0)
            mut = work.tile([4, 512], F32, tag="mut")
            msqt = work.tile([4, 512], F32, tag="msqt")
            mu2 = work.tile([4, 512], F32, tag="mu2")
            rstdt = work.tile([4, 512], BF16, tag="rstdt")
            nmut = work.tile([4, 512], BF16, tag="nmut")
            nc.vector.tensor_scalar_mul(out=mut, in0=ps_sums,
                                        scalar1=1.0 / C)
            nc.vector.tensor_scalar_mul(out=msqt, in0=ps_sq,
                                        scalar1=1.0 / C)
            nc.vector.tensor_mul(out=mu2, in0=mut, in1=mut)
            nc.vector.tensor_sub(out=msqt, in0=msqt, in1=mu2)
            nc.scalar.activation(out=msqt, in_=msqt, func=AF.Sqrt,
                                 bias=eps_t, scale=1.0)
            with nc.allow_low_precision(reason="bf16 rstd"):
                nc.vector.reciprocal(out=rstdt, in_=msqt)
            with nc.allow_low_precision(reason="bf16 mu*rstd"):
                nc.vector.tensor_mul(out=nmut, in0=mut, in1=rstdt)
            return rstdt, nmut

        def ln_pass2(get_src, dst, ntb, psum, wbig, rstdt, nmut):
            # pass 2: broadcast via selector matmul + normalize in-place
            for tb in range(ntb):
                src = get_src(tb)
                sl = slice(tb * 512, (tb + 1) * 512)
                ps_bc = psum.tile([128, 2, 512], F32, tag="bc")
                mm(ps_bc[:, 0, :], selln[:, tb, :], rstdt, True, True)
                mm(ps_bc[:, 1, :], selln[:, tb, :], nmut, True, True)
                bc = wbig.tile([128, 2, 512], BF16, tag="bc")
                with nc.allow_low_precision(reason="bf16 LN broadcast"):
                    nc.vector.tensor_copy(out=bc, in_=ps_bc)
                with nc.allow_low_precision(reason="bf16 LN scale"):
                    nc.vector.tensor_mul(
                        out=dst[:, :, sl], in0=src,
                        in1=bc[:, 0:1, :].broadcast_to([128, NKC, 512]))
                with nc.allow_low_precision(reason="bf16 LN shift"):
                    nc.vector.tensor_sub(
                        out=dst[:, :, sl], in0=dst[:, :, sl],
                        in1=bc[:, 1:2, :].broadcast_to([128, NKC, 512]))

        def layer_norm(get_src, dst, ntb, psum, work, wbig):
            rstdt, nmut = ln_pass1(get_src, ntb, psum, work, wbig)
            ln_pass2(get_src, dst, ntb, psum, wbig, rstdt, nmut)

        # x2 (attention-block residual output) outlives kvp, so open it
        # first; kt/vt/q free before the MLP needs SBUF for m1t.
        x2_cm = tc.tile_pool(name="x2p", bufs=1)
        x2p = x2_cm.__enter__()
        x2t = x2p.tile([128, NKC, OWN], BF16)

        kv_cm = tc.tile_pool(name="kvp", bufs=1)
        kvp = kv_cm.__enter__()
        kt = kvp.tile([128, NKC, T], BF16)             # K, head-pair rows
        vt = kvp.tile([128, NVCH, H, 65], BF16)        # V + ones row
        q_sb = kvp.tile([128, 8, OWN], BF16)           # Q, head-pair rows
        nc.vector.memset(vt[:, :, :, 64:65], 1.0)

        # ======== Phase 1: LN1 over all 2048 tokens ========
        lnx_cm = tc.tile_pool(name="lnxp", bufs=1)
        lnxp = lnx_cm.__enter__()
        lnx = lnxp.tile([128, NKC, T], BF16)

        xt_cm = tc.tile_pool(name="xtp", bufs=2)
        xtp = xt_cm.__enter__()

        def ln1_src(tb):
            xtb = xtp.tile([128, NKC, 512], BF16, tag="xtb")
            nc.sync.dma_start(out=xtb,
                              in_=xt_d[:, :, tb * 512:(tb + 1) * 512])
            return xtb

        w1b_cm = tc.tile_pool(name="ln1wb", bufs=3)
        w1b = w1b_cm.__enter__()
        w1_cm = tc.tile_pool(name="ln1w", bufs=2)
        w1 = w1_cm.__enter__()
        ps1_cm = tc.tile_pool(name="ln1ps", bufs=2, space="PSUM")
        ps1 = ps1_cm.__enter__()
        # two half-calls, pass1s first: the second half's sum-matmuls
        # keep the PE fed while the first half's stats chain runs
        src_b = lambda tb: ln1_src(tb + 2)
        ra = ln_pass1(ln1_src, 2, ps1, w1, w1b)
        load_late_consts()
        rb = ln_pass1(src_b, 2, ps1, w1, w1b)
        ln_pass2(ln1_src, lnx, 2, ps1, w1b, *ra)
        ln_pass2(src_b, lnx[:, :, 1024:2048], 2, ps1, w1b, *rb)
        ps1_cm.__exit__(None, None, None)
        w1_cm.__exit__(None, None, None)
        w1b_cm.__exit__(None, None, None)
        xt_cm.__exit__(None, None, None)

        # ======== Phase 2: QKV projections ========
        qw_cm = tc.tile_pool(name="qkvw", bufs=3)
        qw = qw_cm.__enter__()
        psq_cm = tc.tile_pool(name="qkvps", bufs=6, space="PSUM")
        psq = psq_cm.__enter__()

        # Q (own tokens = local [0:1024)), scaled by 1/sqrt(D)
        for oc in range(8):
            wt = qw.tile([128, NKC, 128], BF16, tag="w")
            nc.sync.dma_start(out=wt, in_=wq_d[oc])
            for tb in range(2):
                sl = slice(tb * 512, (tb + 1) * 512)
                ps = psq.tile([128, 512], F32, tag="mm")
                for k in range(NKC):
                    mm(ps, wt[:, k, :], lnx[:, k, sl], k == 0, k == NKC - 1)
                with nc.allow_low_precision(reason="bf16 q"):
                    nc.vector.tensor_scalar(
                        out=q_sb[:, oc, sl], in0=ps,
                        scalar1=bqt[:, oc:oc + 1], scalar2=1.0 / np.sqrt(D),
                        op0=OP.add, op1=OP.mult)
        # K (all tokens)
        for oc in range(8):
            wt = qw.tile([128, NKC, 128], BF16, tag="w")
            nc.sync.dma_start(out=wt, in_=wk_d[oc])
            for tb in range(4):
                sl = slice(tb * 512, (tb + 1) * 512)
                ps = psq.tile([128, 512], F32, tag="mm")
                for k in range(NKC):
                    mm(ps, wt[:, k, :], lnx[:, k, sl], k == 0, k == NKC - 1)
                with nc.allow_low_precision(reason="bf16 k"):
                    nc.vector.tensor_scalar(
                        out=kt[:, oc, sl], in0=ps,
                        scalar1=bkt[:, oc:oc + 1], scalar2=1.0,
                        op0=OP.add, op1=OP.mult)
        # V (all tokens, token-major), bias via rank-1 matmul
        for g in range(2):
            wv = qw.tile([128, NKC, 512], BF16, tag="wv")
            nc.sync.dma_start(out=wv, in_=wv_d[g])
            for cch in range(NVCH):
                ps = psq.tile([128, 512], F32, tag="mm")
                for k in range(NKC):
                    mm(ps, lnx[:, k, cch * 128:(cch + 1) * 128], wv[:, k, :],
                       k == 0, k == NKC - 1 and not with_vbias)
                if with_vbias:
                    mm(ps, onesrow, bvt[0:1, g, :], False, True)
                with nc.allow_low_precision(reason="bf16 v"):
                    nc.vector.tensor_copy(
                        out=vt[:, cch, g * 8:(g + 1) * 8, 0:64], in_=ps)

        psq_cm.__exit__(None, None, None)
        qw_cm.__exit__(None, None, None)
        lnx_cm.__exit__(None, None, None)

        # ======== Phase 3: attention ========
        y_cm = tc.tile_pool(name="yp", bufs=1)
        yp = y_cm.__enter__()
        y_sb = yp.tile([128, NKC, OWN], BF16)
        mt = yp.tile([128, 16, 512], BF16)             # causal masks
        nc.sync.dma_start(out=mt, in_=masks_d[:, :, :])
        xqf_t = yp.tile([128, NKC, OWN], F32)
        for oc in range(8):
            nc.sync.dma_start(out=xqf_t[:, oc, :], in_=xqf_d[:, oc, :])

        prw_cm = tc.tile_pool(name="prw", bufs=3)
        prw = prw_cm.__enter__()
        attw_cm = tc.tile_pool(name="attw", bufs=5)
        attw = attw_cm.__enter__()
        attsm_cm = tc.tile_pool(name="attsm", bufs=1)
        attsm = attsm_cm.__enter__()
        pss_cm = tc.tile_pool(name="attps", bufs=3, space="PSUM")
        pss = pss_cm.__enter__()
        psy_cm = tc.tile_pool(name="attpy", bufs=1, space="PSUM")
        psy = psy_cm.__enter__()

        # proj is emitted interleaved with attention so the qb1
        # softmax-tail latency hides under proj matmuls; psum comes
        # from the scores pool.
        def proj_tb(tb):
            sl = slice(tb * 512, (tb + 1) * 512)
            for oc in range(8):
                wt = prw.tile([128, NKC, 128], BF16, tag="w")
                nc.sync.dma_start(out=wt, in_=wp_d[oc])
                ps = pss.tile([128, 2, 512], F32, tag="s")
                for k in range(NKC):
                    mm(ps[:, 0, :], wt[:, k, :], y_sb[:, k, sl],
                       k == 0, k == NKC - 1)
                with nc.allow_low_precision(reason="bf16 x2"):
                    nc.vector.scalar_tensor_tensor(
                        out=x2t[:, oc, sl], in0=ps[:, 0, :],
                        scalar=bpt[:, oc:oc + 1],
                        in1=xqf_t[:, oc, sl], op0=OP.add, op1=OP.add)

        # y_sb first holds UNNORMALIZED head outputs. Softmax
        # denominators collect on partition 0 (engine partition-base
        # rules), one SBUF->SBUF DMA scatters them across 16
        # partitions, then a single batched reciprocal + selector
        # matmul broadcasts normalize y. qb0's tail hides under qb1.
        for qb in range(2):
            chunks = CH_QB0 if qb == 0 else CH_QB1
            qsl = slice(qb * 512, (qb + 1) * 512)
            nch = len(chunks)
            dtmp = attsm.tile([1, 16, 512], BF16, tag="dtmp")
            for hp in range(8):
                ha, hb = 2 * hp, 2 * hp + 1
                ps_ya = psy.tile([65, 512], F32, tag="ya")
                ps_yb = psy.tile([65, 512], F32, tag="yb")
                # software pipeline: scores run one chunk ahead of attV
                ps_list = [None] * nch
                ptm_list = [None] * nch

                def scores(i):
                    ci = chunks[i]
                    csl = slice(ci * 128, (ci + 1) * 128)
                    if qb == 0:
                        slot = i
                    else:
                        slot = 8 + QB1_MASKED.index(ci) if ci in QB1_MASKED \
                            else None
                    ps_s = pss.tile([128, 2, 512], F32, tag="s")
                    for j, psl in ((0, slice(0, 64)), (1, slice(64, 128))):
                        mm(ps_s[:, j, :], kt[psl, hp, csl], q_sb[psl, hp, qsl],
                           True, slot is None)
                    if slot is not None:
                        # causal mask as -60000 * maskc accumulated
                        # into the scores via a diagonal matmul
                        for j in range(2):
                            mm(ps_s[:, j, :], ndg, mt[:, slot, :],
                               False, True)
                    ps_list[i] = ps_s

                def softmax(i):
                    pt = attw.tile([128, 2, 512], BF16, tag="pt")
                    with nc.allow_low_precision(reason="bf16 softmax"):
                        nc.scalar.activation(out=pt, in_=ps_list[i],
                                             func=AF.Exp)
                    ptm_list[i] = pt

                def attv(i):
                    ci = chunks[i]
                    ptm = ptm_list[i]
                    mm(ps_ya, vt[:, ci, ha, :], ptm[:, 0, :],
                       i == 0, i == nch - 1)
                    mm(ps_yb, vt[:, ci, hb, :], ptm[:, 1, :],
                       i == 0, i == nch - 1)
                    ps_list[i] = None
                    ptm_list[i] = None

                for i in range(3):
                    scores(i)
                    softmax(i)
                for i in range(3, nch):
                    scores(i)
                    softmax(i)
                    attv(i - 3)
                for i in range(nch - 3, nch):
                    attv(i)

                for h, ps_y in ((ha, ps_ya), (hb, ps_yb)):
                    with nc.allow_low_precision(reason="bf16 y raw"):
                        nc.vector.tensor_copy(
                            out=y_sb[(h % 2) * 64:(h % 2) * 64 + 64,
                                     h // 2, qsl],
                            in_=ps_y[0:64, :])
                    hi = 2 * hp + h % 2
                    with nc.allow_low_precision(reason="bf16 denom"):
                        nc.vector.tensor_copy(out=dtmp[0:1, hi, :],
                                              in_=ps_y[64:65, :])

            # per-qb normalize tail: selector-matmul broadcasts both
            # heads' reciprocal rows into one [128, 512] tile per pair
            den16 = attsm.tile([16, 512], BF16, tag="den16")
            nc.sync.dma_start(out=den16, in_=dtmp[0:1, :, :])
            rden = attsm.tile([16, 512], BF16, tag="rden")
            with nc.allow_low_precision(reason="bf16 softmax denom"):
                nc.vector.reciprocal(out=rden, in_=den16)
            if qb == 1:
                # keep the PE fed while the qb1 denominator DMA +
                # reciprocal complete
                proj_tb(0)
            for hp in range(8):
                rb = pss.tile([128, 2, 512], F32, tag="s")
                mm(rb[:, 0, :], selt[:, hp, :], rden, True, True)
                ysl = y_sb[:, hp, qsl]
                with nc.allow_low_precision(reason="bf16 y norm"):
                    nc.vector.tensor_mul(out=ysl, in0=ysl, in1=rb[:, 0, :])
        proj_tb(1)

        psy_cm.__exit__(None, None, None)
        pss_cm.__exit__(None, None, None)
        attsm_cm.__exit__(None, None, None)
        attw_cm.__exit__(None, None, None)
        prw_cm.__exit__(None, None, None)
        y_cm.__exit__(None, None, None)
        kv_cm.__exit__(None, None, None)

        # ======== Phase 5: LN2 ========
        ln2x_cm = tc.tile_pool(name="ln2xp", bufs=1)
        ln2xp = ln2x_cm.__enter__()
        ln2x = ln2xp.tile([128, NKC, OWN], BF16)
        w2b_cm = tc.tile_pool(name="ln2wb", bufs=2)
        w2b = w2b_cm.__enter__()
        w2_cm = tc.tile_pool(name="ln2w", bufs=1)
        w2 = w2_cm.__enter__()
        ps2_cm = tc.tile_pool(name="ln2ps", bufs=2, space="PSUM")
        ps2 = ps2_cm.__enter__()
        layer_norm(lambda tb: x2t[:, :, tb * 512:(tb + 1) * 512],
                   ln2x, OWN // 512, ps2, w2, w2b)
        ps2_cm.__exit__(None, None, None)
        w2_cm.__exit__(None, None, None)
        w2b_cm.__exit__(None, None, None)

        # ======== Phase 6: MLP ========
        mlp_cm = tc.tile_pool(name="mlpp", bufs=1)
        mlpp = mlp_cm.__enter__()
        m1t = mlpp.tile([128, NFFC, OWN], BF16)
        mw1_cm = tc.tile_pool(name="mw1", bufs=3)
        mw1 = mw1_cm.__enter__()
        mw2_cm = tc.tile_pool(name="mw2", bufs=2)
        mw2 = mw2_cm.__enter__()
        mo_cm = tc.tile_pool(name="mo", bufs=3)
        mo = mo_cm.__enter__()
        psm_cm = tc.tile_pool(name="mlpps", bufs=4, space="PSUM")
        psm = psm_cm.__enter__()

        # tb-outer: all of block 0's fc1 runs before anything needs
        # ln2x block 1, hiding the LN2 stats tail (wf1 loads twice;
        # the extra 8MB of DMA hides under 55us of matmuls)
        for tb in range(2):
            sl = slice(tb * 512, (tb + 1) * 512)
            for ffc in range(NFFC):
                wt = mw1.tile([128, NKC, 128], BF16, tag="w1")
                nc.sync.dma_start(out=wt, in_=wf1_d[ffc])
                ps = psm.tile([128, 512], F32, tag="mm1")
                for k in range(NKC):
                    mm(ps, wt[:, k, :], ln2x[:, k, sl], k == 0, k == NKC - 1)
                with nc.allow_low_precision(reason="bf16 mlp hidden"):
                    nc.vector.tensor_scalar(
                        out=m1t[:, ffc, sl], in0=ps,
                        scalar1=bf1t[:, ffc:ffc + 1], scalar2=0.0,
                        op0=OP.add, op1=OP.max)
        for oc in range(NKC):
            wt2 = mw2.tile([128, NFFC, 128], BF16, tag="w2")
            nc.sync.dma_start(out=wt2, in_=wf2_d[oc])
            for tb in range(2):
                sl = slice(tb * 512, (tb + 1) * 512)
                ps = psm.tile([128, 512], F32, tag="mm2")
                for k in range(NFFC):
                    mm(ps, wt2[:, k, :], m1t[:, k, sl], k == 0, k == NFFC - 1)
                ot = mo.tile([128, 512], F32, tag="ot")
                nc.vector.scalar_tensor_tensor(
                    out=ot, in0=ps, scalar=bf2t[:, oc:oc + 1],
                    in1=x2t[:, oc, sl], op0=OP.add, op1=OP.add)
                nc.sync.dma_start(out=out_d[:, oc, sl], in_=ot)

        psm_cm.__exit__(None, None, None)
        mo_cm.__exit__(None, None, None)
        mw2_cm.__exit__(None, None, None)
        mw1_cm.__exit__(None, None, None)
        mlp_cm.__exit__(None, None, None)
        ln2x_cm.__exit__(None, None, None)
        x2_cm.__exit__(None, None, None)
        consts_cm.__exit__(None, None, None)

    nc.compile()
    return nc


class _SpmdRunner:
    def __init__(self, nc, n_cores=NC):
        import jax
        from jax.sharding import Mesh, PartitionSpec
        from jax.experimental.shard_map import shard_map
        import concourse.mybir as mybir
        from concourse import bass2jax
        bass2jax.install_neuronx_cc_hook()
        self.jax = jax
        self.n_cores = n_cores
        partition_name = (
            nc.partition_id_tensor.name if nc.partition_id_tensor else None)
        in_names, out_names, out_avals = [], [], []
        for alloc in nc.m.functions[0].allocations:
            if not isinstance(alloc, mybir.MemoryLocationSet):
                continue
            name = alloc.memorylocations[0].name
            if alloc.kind == "ExternalInput":
                if name != partition_name:
                    in_names.append(name)
            elif alloc.kind == "ExternalOutput":
                out_names.append(name)
                out_avals.append(jax.core.ShapedArray(
                    tuple(alloc.tensor_shape), mybir.dt.np(alloc.dtype)))
        self.in_names = in_names
        self.out_names = out_names
        self.out_avals = out_avals
        all_in = in_names + out_names
        if partition_name is not None:
            all_in.append(partition_name)

        def _body(*args):
            operands = list(args)
            if partition_name is not None:
                operands.append(bass2jax.partition_id_tensor())
            outs = bass2jax._bass_exec_p.bind(
                *operands, out_avals=tuple(out_avals),
                in_names=tuple(all_in), out_names=tuple(out_names),
                lowering_input_output_aliases=(),
                sim_require_finite=True, sim_require_nnan=True, nc=nc)
            return tuple(outs)

        devices = jax.devices()[:n_cores]
        self.mesh = Mesh(np.asarray(devices), ("core",))
        n_io = len(in_names) + len(out_names)
        self.fn = jax.jit(
            shard_map(_body, mesh=self.mesh,
                      in_specs=(PartitionSpec("core"),) * n_io,
                      out_specs=(PartitionSpec("core"),) * len(out_names),
                      check_rep=False),
            keep_unused=True)
        self._dev_in = None

    def put_inputs(self, in_maps):
        from jax.sharding import NamedSharding, PartitionSpec
        jax = self.jax
        sh = NamedSharding(self.mesh, PartitionSpec("core"))
        concat = []
        for name in self.in_names:
            arrs = [np.asarray(in_maps[c][name]) for c in range(self.n_cores)]
            concat.append(jax.device_put(np.concatenate(arrs, axis=0), sh))
        for av in self.out_avals:
            z = np.zeros((self.n_cores * av.shape[0], *av.shape[1:]), av.dtype)
            concat.append(jax.device_put(z, sh))
        self._dev_in = concat

    def run(self):
        jax = self.jax
        outs = self.fn(*self._dev_in)
        jax.block_until_ready(outs)
        results = []
        for c in range(self.n_cores):
            d = {}
            for i, name in enumerate(self.out_names):
                av = self.out_avals[i]
                d[name] = np.asarray(outs[i]).reshape(
                    self.n_cores, *av.shape)[c]
            results.append(d)
        return results

    def time_exec(self, warmup=3, m1=4, m2=12, reps=3, trials=6):
        """Estimate per-call device time by dispatching bursts of m1 and
        m2 back-to-back calls and differencing, which cancels the
        constant dispatch/RTT overhead of the axon tunnel. Dispatch
        stalls only ever inflate a burst, so the minimum over several
        trials is the tightest estimate of true device throughput."""
        import time
        jax = self.jax
        for _ in range(warmup):
            jax.block_until_ready(self.fn(*self._dev_in))

        def burst(m):
            t0 = time.perf_counter()
            outs = None
            for _ in range(m):
                outs = self.fn(*self._dev_in)
            jax.block_until_ready(outs)
            return time.perf_counter() - t0

        t1s, t2s = [], []
        for _ in range(trials):
            for _ in range(reps):
                t1s.append(burst(m1))
                t2s.append(burst(m2))
        est = (min(t2s) - min(t1s)) / (m2 - m1)
        if est <= 0:
            # dispatch noise overwhelmed the diff; fall back to the
            # tightest whole-burst bound (includes per-call overhead)
            est = min(min(t2s) / m2, min(t1s) / m1)
        return est


def _get_runner(with_vbias=None):
    if with_vbias is None:
        if "last" in _STATE:
            return _STATE["last"]
        with_vbias = True
    key = ("runner", with_vbias)
    if key not in _STATE:
        nc = _build_program(with_vbias)
        _STATE[key] = _SpmdRunner(nc)
    _STATE["last"] = _STATE[key]
    return _STATE[key]


def _perm(r):
    """Per-core token permutation: own query tokens first (zigzag)."""
    if r == 0:
        return np.concatenate([np.arange(0, 512), np.arange(1536, 2048),
                               np.arange(512, 1536)])
    return np.concatenate([np.arange(512, 1536), np.arange(0, 512),
                           np.arange(1536, 2048)])


def _prep_in_maps(x, W_attn, W_proj, b_proj, W_fc1, b_fc1, W_fc2, b_fc2,
                  ln1_g, ln1_b, ln2_g, ln2_b):
    f32 = np.float32
    bf16 = ml_dtypes.bfloat16
    x = np.asarray(x, f32)
    W_attn = np.asarray(W_attn, f32)
    g1 = np.asarray(ln1_g, f32)
    b1 = np.asarray(ln1_b, f32)
    g2 = np.asarray(ln2_g, f32)
    b2 = np.asarray(ln2_b, f32)
    Wq = g1[:, None] * W_attn[:, 0:C]
    Wk = g1[:, None] * W_attn[:, C:2 * C]
    Wv = g1[:, None] * W_attn[:, 2 * C:3 * C]
    bq = b1 @ W_attn[:, 0:C]
    bk = b1 @ W_attn[:, C:2 * C]
    bv = b1 @ W_attn[:, 2 * C:3 * C]
    Wfc1 = g2[:, None] * np.asarray(W_fc1, f32)
    bfc1 = np.asarray(b_fc1, f32) + b2 @ np.asarray(W_fc1, f32)

    def lhs_tiles(W, nout):
        # [C, nout*128] -> [nout, 128p, NKC, 128m]
        return np.ascontiguousarray(
            W.reshape(NKC, 128, nout, 128).transpose(2, 1, 0, 3)).astype(bf16)

    wq = lhs_tiles(Wq, 8)
    wk = lhs_tiles(Wk, 8)
    wv = np.ascontiguousarray(
        Wv.reshape(NKC, 128, 2, 512).transpose(2, 1, 0, 3)).astype(bf16)
    wp = lhs_tiles(np.asarray(W_proj, f32), 8)
    wf1 = lhs_tiles(Wfc1, NFFC)
    wf2 = np.ascontiguousarray(
        np.asarray(W_fc2, f32).reshape(NFFC, 128, NKC, 128)
        .transpose(2, 1, 0, 3)).astype(bf16)

    def vec(v, nk):
        return np.ascontiguousarray(np.asarray(v, f32).reshape(nk, 128).T)

    sel = np.zeros((16, 8, 128), f32)
    for hp in range(8):
        sel[2 * hp, hp, 0:64] = 1.0
        sel[2 * hp + 1, hp, 64:128] = 1.0
    # LN selectors: ones4 routes block tb's sum to psum row tb; selln
    # broadcasts rnt row tb (rstd, slot 2tb) / row 32+tb (mu*rstd,
    # slot 2tb+1) across all 128 partitions
    ones4 = np.zeros((128, 4, 4), f32)
    for tb in range(4):
        ones4[:, tb, tb] = 1.0
    selln = np.zeros((4, 4, 128), f32)
    for tb in range(4):
        selln[tb, tb, :] = 1.0

    shared = {
        "wq": wq, "wk": wk, "wv": wv, "wp": wp, "wf1": wf1, "wf2": wf2,
        "sel": sel.astype(bf16),
        "ndg": (-60000.0 * np.eye(128, dtype=f32)).astype(bf16),
        "ones4": ones4.astype(bf16),
        "selln": selln.astype(bf16),
        "bq": vec(bq, 8), "bk": vec(bk, 8),
        "bv": np.ascontiguousarray(bv.reshape(1, 2, 512)).astype(bf16),
        "bp": vec(np.asarray(b_proj, f32), NKC),
        "bf1": vec(bfc1, NFFC),
        "bf2": vec(np.asarray(b_fc2, f32), NKC),
    }

    in_maps = []
    for c in range(NC):
        b, r = c // 2, c % 2
        perm = _perm(r)
        xp = x[b][perm]                       # [T, C] local token order
        xt = np.ascontiguousarray(
            xp.T.reshape(NKC, 128, T).transpose(1, 0, 2)).astype(bf16)
        xqf = np.ascontiguousarray(
            xp[:OWN].T.reshape(NKC, 128, OWN).transpose(1, 0, 2))
        # masks[p, slot, qi]: slots 0..7 = qb0 chunks CH_QB0;
        # slots 8..15 = qb1 chunks QB1_MASKED. 1 where kv_g <= q_g.
        # complement masks: 1 where attention is FORBIDDEN (kv > q)
        m = np.zeros((128, 16, 512), f32)
        for j, ci in enumerate(CH_QB0):
            gkv = perm[ci * 128:(ci + 1) * 128]
            gq = perm[0:512]
            m[:, j, :] = (gkv[:, None] > gq[None, :]).astype(f32)
        for j, ci in enumerate(QB1_MASKED):
            gkv = perm[ci * 128:(ci + 1) * 128]
            gq = perm[512:1024]
            m[:, 8 + j, :] = (gkv[:, None] > gq[None, :]).astype(f32)
        d = {"xt": xt, "xqf": xqf, "masks": m.astype(bf16)}
        d.update(shared)
        in_maps.append(d)
    return in_maps


def kernel(x, W_attn, W_proj, b_proj, W_fc1, b_fc1, W_fc2, b_fc2,
           ln1_g, ln1_b, ln2_g, ln2_b):
    bv = np.asarray(ln1_b, np.float32) @ np.asarray(
        W_attn, np.float32)[:, 2 * C:3 * C]
    runner = _get_runner(bool(np.any(bv != 0.0)))
    in_maps = _prep_in_maps(x, W_attn, W_proj, b_proj, W_fc1, b_fc1,
                            W_fc2, b_fc2, ln1_g, ln1_b, ln2_g, ln2_b)
    runner.put_inputs(in_maps)
    results = runner.run()
    out = np.empty((B, T, C), np.float32)
    for c in range(NC):
        b, r = c // 2, c % 2
        ot = results[c]["out"]                # [128, NKC, OWN]
        feat = ot.transpose(1, 0, 2).reshape(C, OWN)
        out[b, _perm(r)[:OWN], :] = feat.T
    return out


# revision 93
# speedup vs baseline: 4.6741x; 1.3647x over previous
"""Dense transformer block (B=4, T=2048, C=1024, H=16, FF=4096) on 8
Trainium2 NeuronCores.

Sharding: sequence-parallel, zero collectives. Core c handles batch
b = c // 2 and query-token half r = c % 2. The host permutes each
core's tokens so its OWN query tokens occupy local positions [0:1024)
(zigzag assignment: r=0 owns global [0:512)+[1536:2048), r=1 owns
[512:1536)), which removes the separate query-token layer-norm pass.
Each core redundantly computes LN1 + K/V for the full 2048-token
sequence of its batch, so no cross-core communication is needed.
Causality is enforced with per-core mask tensors (input data); the
attention chunk sets are uniform across cores: q-block 0 attends local
kv chunks {0..3, 8..11} (all maskable), q-block 1 attends all 16
chunks (slots 4..7 and 12..15 maskable).

All activations and weights are bf16 (fp32 PSUM accumulation); LN
gains are folded into the weights host-side and LN/QKV biases are
applied as per-partition bias in the PSUM->SBUF copies (V's bias via a
rank-1 accumulating matmul). Everything stays SBUF-resident between
phases - no DRAM bounce buffers.
"""
import numpy as np
import ml_dtypes

B, T, C = 4, 2048, 1024
H, D, FF = 16, 64, 4096
NC = 8
NKC = C // 128     # 8 feature chunks
NFFC = FF // 128   # 32
NVCH = T // 128    # 16 kv chunks
OWN = 1024         # own query tokens per core
EPS = 1e-5

CH_QB0 = [0, 1, 2, 3, 8, 9, 10, 11]          # qb0 chunk set (all masked)
CH_QB1 = list(range(16))                      # qb1 chunk set
QB1_MASKED = [4, 5, 6, 7, 12, 13, 14, 15]     # masked slots of qb1

_STATE = {}


def _build_program(with_vbias=True):
    import concourse.bacc as bacc
    import concourse.mybir as mybir
    from concourse.tile import TileContext

    F32 = mybir.dt.float32
    BF16 = mybir.dt.bfloat16
    AF = mybir.ActivationFunctionType
    OP = mybir.AluOpType

    nc = bacc.Bacc("TRN2", target_bir_lowering=False, debug=False,
                   num_devices=NC)

    xt_d = nc.dram_tensor("xt", [128, NKC, T], BF16, kind="ExternalInput")
    xqf_d = nc.dram_tensor("xqf", [128, NKC, OWN], F32, kind="ExternalInput")
    wq_d = nc.dram_tensor("wq", [8, 128, NKC, 128], BF16, kind="ExternalInput")
    wk_d = nc.dram_tensor("wk", [8, 128, NKC, 128], BF16, kind="ExternalInput")
    wv_d = nc.dram_tensor("wv", [2, 128, NKC, 512], BF16, kind="ExternalInput")
    wp_d = nc.dram_tensor("wp", [8, 128, NKC, 128], BF16, kind="ExternalInput")
    wf1_d = nc.dram_tensor("wf1", [NFFC, 128, NKC, 128], BF16,
                           kind="ExternalInput")
    wf2_d = nc.dram_tensor("wf2", [NKC, 128, NFFC, 128], BF16,
                           kind="ExternalInput")
    bq_d = nc.dram_tensor("bq", [128, 8], F32, kind="ExternalInput")
    bk_d = nc.dram_tensor("bk", [128, 8], F32, kind="ExternalInput")
    bv_d = nc.dram_tensor("bv", [1, 2, 512], BF16, kind="ExternalInput")
    bp_d = nc.dram_tensor("bp", [128, NKC], F32, kind="ExternalInput")
    bf1_d = nc.dram_tensor("bf1", [128, NFFC], F32, kind="ExternalInput")
    bf2_d = nc.dram_tensor("bf2", [128, NKC], F32, kind="ExternalInput")
    masks_d = nc.dram_tensor("masks", [128, 16, 512], BF16,
                             kind="ExternalInput")
    sel_d = nc.dram_tensor("sel", [16, 8, 128], BF16, kind="ExternalInput")
    ndg_d = nc.dram_tensor("ndg", [128, 128], BF16, kind="ExternalInput")
    ones4_d = nc.dram_tensor("ones4", [128, 4, 4], BF16, kind="ExternalInput")
    selln_d = nc.dram_tensor("selln", [4, 4, 128], BF16,
                             kind="ExternalInput")
    out_d = nc.dram_tensor("out", [128, NKC, OWN], F32, kind="ExternalOutput")

    def mm(ps, lhsT, rhs, start, stop):
        nc.tensor.matmul(ps, lhsT, rhs, start=start, stop=stop)

    with TileContext(nc, pool_alloc_mode="queue") as tc:
        consts_cm = tc.tile_pool(name="consts", bufs=1)
        consts = consts_cm.__enter__()

        ones128 = consts.tile([128, 1], BF16)
        nc.vector.memset(ones128, 1.0)
        onesrow = consts.tile([1, 128], BF16)
        nc.vector.memset(onesrow, 1.0)
        eps_t = consts.tile([4, 1], F32)
        nc.vector.memset(eps_t, EPS)
        # only the LN1-critical constants load up front; the rest are
        # emitted after LN1 so the first x-block DMA isn't queued
        # behind them on the sync engine
        ones4 = consts.tile([128, 4, 4], BF16)
        nc.sync.dma_start(out=ones4, in_=ones4_d[:, :, :])
        selln = consts.tile([4, 4, 128], BF16)
        nc.sync.dma_start(out=selln, in_=selln_d[:, :, :])
        bqt = consts.tile([128, 8], F32)
        bkt = consts.tile([128, 8], F32)
        bvt = consts.tile([1, 2, 512], BF16)
        bpt = consts.tile([128, NKC], F32)
        bf1t = consts.tile([128, NFFC], F32)
        bf2t = consts.tile([128, NKC], F32)
        selt = consts.tile([16, 8, 128], BF16)
        ndg = consts.tile([128, 128], BF16)

        def load_late_consts():
            nc.sync.dma_start(out=bqt, in_=bq_d[:, :])
            nc.sync.dma_start(out=bkt, in_=bk_d[:, :])
            nc.sync.dma_start(out=bvt, in_=bv_d[:, :, :])
            nc.sync.dma_start(out=bpt, in_=bp_d[:, :])
            nc.sync.dma_start(out=bf1t, in_=bf1_d[:, :])
            nc.sync.dma_start(out=bf2t, in_=bf2_d[:, :])
            nc.sync.dma_start(out=selt, in_=sel_d[:, :, :])
            nc.sync.dma_start(out=ndg, in_=ndg_d[:, :])

        # ---------------- layer norm over feature dim -----------------
        # dst = (src - mu) * rstd, with LN gain/bias folded into the
        # downstream weights/biases host-side. Stats accumulate via
        # ones-matmuls; squares on GpSimd; rstd via ACT Rsqrt.
        def ln_pass1(get_src, ntb, psum, work, wbig):
            # pass 1: per-block sums land in DISTINCT ROWS of one
            # [4,512] psum tile (selector lhsT columns), so the stats
            # chain runs ONCE, batched across blocks.
            ps_sums = psum.tile([4, 512], F32, tag="s")
            ps_sq = psum.tile([4, 512], F32, tag="q")
            sqs = [None] * ntb

            def sq_mms(tb):
                for k in range(NKC):
                    mm(ps_sq, ones4[:, tb, :], sqs[tb][:, k, :],
                       tb == 0 and k == 0, tb == ntb - 1 and k == NKC - 1)
                sqs[tb] = None

            for tb in range(ntb):
                src = get_src(tb)
                for k in range(NKC):
                    mm(ps_sums, ones4[:, tb, :], src[:, k, :],
                       tb == 0 and k == 0, tb == ntb - 1 and k == NKC - 1)
                sq = wbig.tile([128, NKC, 512], BF16, tag="sq")
                with nc.allow_low_precision(reason="bf16 x^2 for variance"):
                    nc.scalar.activation(out=sq, in_=src, func=AF.Square)
                sqs[tb] = sq
                if tb >= 2:
                    sq_mms(tb - 2)
            for tb in range(max(0, ntb - 2), ntb):
                sq_mms(tb)
            # batched stats across blocks (all tiles base partition 0)
            mut = work.tile([4, 512], F32, tag="mut")
            msqt = work.tile([4, 512], F32, tag="msqt")
            mu2 = work.tile([4, 512], F32, tag="mu2")
            rstdt = work.tile([4, 512], BF16, tag="rstdt")
            nmut = work.tile([4, 512], BF16, tag="nmut")
            nc.vector.tensor_scalar_mul(out=mut, in0=ps_sums,
                                        scalar1=1.0 / C)
            nc.vector.tensor_scalar_mul(out=msqt, in0=ps_sq,
                                        scalar1=1.0 / C)
            nc.vector.tensor_mul(out=mu2, in0=mut, in1=mut)
            nc.vector.tensor_sub(out=msqt, in0=msqt, in1=mu2)
            nc.scalar.activation(out=msqt, in_=msqt, func=AF.Sqrt,
                                 bias=eps_t, scale=1.0)
            with nc.allow_low_precision(reason="bf16 rstd"):
                nc.vector.reciprocal(out=rstdt, in_=msqt)
            with nc.allow_low_precision(reason="bf16 mu*rstd"):
                nc.vector.tensor_mul(out=nmut, in0=mut, in1=rstdt)
            return rstdt, nmut

        def ln_pass2(get_src, dst, ntb, psum, wbig, rstdt, nmut):
            # pass 2: broadcast via selector matmul + normalize in-place
            for tb in range(ntb):
                src = get_src(tb)
                sl = slice(tb * 512, (tb + 1) * 512)
                ps_bc = psum.tile([128, 2, 512], F32, tag="bc")
                mm(ps_bc[:, 0, :], selln[:, tb, :], rstdt, True, True)
                mm(ps_bc[:, 1, :], selln[:, tb, :], nmut, True, True)
                bc = wbig.tile([128, 2, 512], BF16, tag="bc")
                with nc.allow_low_precision(reason="bf16 LN broadcast"):
                    nc.vector.tensor_copy(out=bc, in_=ps_bc)
                with nc.allow_low_precision(reason="bf16 LN scale"):
                    nc.vector.tensor_mul(
                        out=dst[:, :, sl], in0=src,
                        in1=bc[:, 0:1, :].broadcast_to([128, NKC, 512]))
                with nc.allow_low_precision(reason="bf16 LN shift"):
                    nc.vector.tensor_sub(
                        out=dst[:, :, sl], in0=dst[:, :, sl],
                        in1=bc[:, 1:2, :].broadcast_to([128, NKC, 512]))

        def layer_norm(get_src, dst, ntb, psum, work, wbig):
            rstdt, nmut = ln_pass1(get_src, ntb, psum, work, wbig)
            ln_pass2(get_src, dst, ntb, psum, wbig, rstdt, nmut)

        # x2 (attention-block residual output) outlives kvp, so open it
        # first; kt/vt/q free before the MLP needs SBUF for m1t.
        x2_cm = tc.tile_pool(name="x2p", bufs=1)
        x2p = x2_cm.__enter__()
        x2t = x2p.tile([128, NKC, OWN], BF16)

        kv_cm = tc.tile_pool(name="kvp", bufs=1)
        kvp = kv_cm.__enter__()
        kt = kvp.tile([128, NKC, T], BF16)             # K, head-pair rows
        vt = kvp.tile([128, NVCH, H, 65], BF16)        # V + ones row
        q_sb = kvp.tile([128, 8, OWN], BF16)           # Q, head-pair rows
        nc.vector.memset(vt[:, :, :, 64:65], 1.0)

        # ======== Phase 1: LN1 over all 2048 tokens ========
        lnx_cm = tc.tile_pool(name="lnxp", bufs=1)
        lnxp = lnx_cm.__enter__()
        lnx = lnxp.tile([128, NKC, T], BF16)

        xt_cm = tc.tile_pool(name="xtp", bufs=2)
        xtp = xt_cm.__enter__()

        def ln1_src(tb):
            xtb = xtp.tile([128, NKC, 512], BF16, tag="xtb")
            nc.sync.dma_start(out=xtb,
                              in_=xt_d[:, :, tb * 512:(tb + 1) * 512])
            return xtb

        w1b_cm = tc.tile_pool(name="ln1wb", bufs=3)
        w1b = w1b_cm.__enter__()
        w1_cm = tc.tile_pool(name="ln1w", bufs=2)
        w1 = w1_cm.__enter__()
        ps1_cm = tc.tile_pool(name="ln1ps", bufs=2, space="PSUM")
        ps1 = ps1_cm.__enter__()
        # two half-calls, pass1s first: the second half's sum-matmuls
        # keep the PE fed while the first half's stats chain runs
        src_b = lambda tb: ln1_src(tb + 2)
        ra = ln_pass1(ln1_src, 2, ps1, w1, w1b)
        load_late_consts()
        rb = ln_pass1(src_b, 2, ps1, w1, w1b)
        ln_pass2(ln1_src, lnx, 2, ps1, w1b, *ra)
        ln_pass2(src_b, lnx[:, :, 1024:2048], 2, ps1, w1b, *rb)
        ps1_cm.__exit__(None, None, None)
        w1_cm.__exit__(None, None, None)
        w1b_cm.__exit__(None, None, None)
        xt_cm.__exit__(None, None, None)

        # ======== Phase 2: QKV projections ========
        qw_cm = tc.tile_pool(name="qkvw", bufs=3)
        qw = qw_cm.__enter__()
        psq_cm = tc.tile_pool(name="qkvps", bufs=6, space="PSUM")
        psq = psq_cm.__enter__()

        # Q (own tokens = local [0:1024)), scaled by 1/sqrt(D)
        for oc in range(8):
            wt = qw.tile([128, NKC, 128], BF16, tag="w")
            nc.sync.dma_start(out=wt, in_=wq_d[oc])
            for tb in range(2):
                sl = slice(tb * 512, (tb + 1) * 512)
                ps = psq.tile([128, 512], F32, tag="mm")
                for k in range(NKC):
                    mm(ps, wt[:, k, :], lnx[:, k, sl], k == 0, k == NKC - 1)
                with nc.allow_low_precision(reason="bf16 q"):
                    nc.vector.tensor_scalar(
                        out=q_sb[:, oc, sl], in0=ps,
                        scalar1=bqt[:, oc:oc + 1], scalar2=1.0 / np.sqrt(D),
                        op0=OP.add, op1=OP.mult)
        # K (all tokens)
        for oc in range(8):
            wt = qw.tile([128, NKC, 128], BF16, tag="w")
            nc.sync.dma_start(out=wt, in_=wk_d[oc])
            for tb in range(4):
                sl = slice(tb * 512, (tb + 1) * 512)
                ps = psq.tile([128, 512], F32, tag="mm")
                for k in range(NKC):
                    mm(ps, wt[:, k, :], lnx[:, k, sl], k == 0, k == NKC - 1)
                with nc.allow_low_precision(reason="bf16 k"):
                    nc.vector.tensor_scalar(
                        out=kt[:, oc, sl], in0=ps,
                        scalar1=bkt[:, oc:oc + 1], scalar2=1.0,
                        op0=OP.add, op1=OP.mult)
        # V (all tokens, token-major), bias via rank-1 matmul
        for g in range(2):
            wv = qw.tile([128, NKC, 512], BF16, tag="wv")
            nc.sync.dma_start(out=wv, in_=wv_d[g])
            for cch in range(NVCH):
                ps = psq.tile([128, 512], F32, tag="mm")
                for k in range(NKC):
                    mm(ps, lnx[:, k, cch * 128:(cch + 1) * 128], wv[:, k, :],
                       k == 0, k == NKC - 1 and not with_vbias)
                if with_vbias:
                    mm(ps, onesrow, bvt[0:1, g, :], False, True)
                with nc.allow_low_precision(reason="bf16 v"):
                    nc.vector.tensor_copy(
                        out=vt[:, cch, g * 8:(g + 1) * 8, 0:64], in_=ps)

        psq_cm.__exit__(None, None, None)
        qw_cm.__exit__(None, None, None)
        lnx_cm.__exit__(None, None, None)

        # ======== Phase 3: attention ========
        y_cm = tc.tile_pool(name="yp", bufs=1)
        yp = y_cm.__enter__()
        y_sb = yp.tile([128, NKC, OWN], BF16)
        mt = yp.tile([128, 16, 512], BF16)             # causal masks
        nc.sync.dma_start(out=mt, in_=masks_d[:, :, :])
        xqf_t = yp.tile([128, NKC, OWN], F32)
        for oc in range(8):
            nc.sync.dma_start(out=xqf_t[:, oc, :], in_=xqf_d[:, oc, :])

        prw_cm = tc.tile_pool(name="prw", bufs=3)
        prw = prw_cm.__enter__()
        attw_cm = tc.tile_pool(name="attw", bufs=5)
        attw = attw_cm.__enter__()
        attsm_cm = tc.tile_pool(name="attsm", bufs=1)
        attsm = attsm_cm.__enter__()
        pss_cm = tc.tile_pool(name="attps", bufs=3, space="PSUM")
        pss = pss_cm.__enter__()
        psy_cm = tc.tile_pool(name="attpy", bufs=1, space="PSUM")
        psy = psy_cm.__enter__()

        # proj is emitted interleaved with attention so the qb1
        # softmax-tail latency hides under proj matmuls; psum comes
        # from the scores pool.
        def proj_tb(tb):
            sl = slice(tb * 512, (tb + 1) * 512)
            for oc in range(8):
                wt = prw.tile([128, NKC, 128], BF16, tag="w")
                nc.sync.dma_start(out=wt, in_=wp_d[oc])
                ps = pss.tile([128, 2, 512], F32, tag="s")
                for k in range(NKC):
                    mm(ps[:, 0, :], wt[:, k, :], y_sb[:, k, sl],
                       k == 0, k == NKC - 1)
                with nc.allow_low_precision(reason="bf16 x2"):
                    nc.vector.scalar_tensor_tensor(
                        out=x2t[:, oc, sl], in0=ps[:, 0, :],
                        scalar=bpt[:, oc:oc + 1],
                        in1=xqf_t[:, oc, sl], op0=OP.add, op1=OP.add)

        # y_sb first holds UNNORMALIZED head outputs. Softmax
        # denominators collect on partition 0 (engine partition-base
        # rules), one SBUF->SBUF DMA scatters them across 16
        # partitions, then a single batched reciprocal + selector
        # matmul broadcasts normalize y. qb0's tail hides under qb1.
        pending = []
        for qb in range(2):
            chunks = CH_QB0 if qb == 0 else CH_QB1
            qsl = slice(qb * 512, (qb + 1) * 512)
            nch = len(chunks)
            dtmp = attsm.tile([1, 16, 512], BF16, tag="dtmp")
            for hp in range(8):
                if qb == 1 and hp == 1 and pending:
                    pending.pop()()
                ha, hb = 2 * hp, 2 * hp + 1
                ps_ya = psy.tile([65, 512], F32, tag="ya")
                ps_yb = psy.tile([65, 512], F32, tag="yb")
                # software pipeline: scores run one chunk ahead of attV
                ps_list = [None] * nch
                ptm_list = [None] * nch

                def scores(i):
                    ci = chunks[i]
                    csl = slice(ci * 128, (ci + 1) * 128)
                    if qb == 0:
                        slot = i
                    else:
                        slot = 8 + QB1_MASKED.index(ci) if ci in QB1_MASKED \
                            else None
                    ps_s = pss.tile([128, 2, 512], F32, tag="s")
                    for j, psl in ((0, slice(0, 64)), (1, slice(64, 128))):
                        mm(ps_s[:, j, :], kt[psl, hp, csl], q_sb[psl, hp, qsl],
                           True, slot is None)
                    if slot is not None:
                        # causal mask as -60000 * maskc accumulated
                        # into the scores via a diagonal matmul
                        for j in range(2):
                            mm(ps_s[:, j, :], ndg, mt[:, slot, :],
                               False, True)
                    ps_list[i] = ps_s

                def softmax(i):
                    pt = attw.tile([128, 2, 512], BF16, tag="pt")
                    with nc.allow_low_precision(reason="bf16 softmax"):
                        nc.scalar.activation(out=pt, in_=ps_list[i],
                                             func=AF.Exp)
                    ptm_list[i] = pt

                def attv(i):
                    ci = chunks[i]
                    ptm = ptm_list[i]
                    mm(ps_ya, vt[:, ci, ha, :], ptm[:, 0, :],
                       i == 0, i == nch - 1)
                    mm(ps_yb, vt[:, ci, hb, :], ptm[:, 1, :],
                       i == 0, i == nch - 1)
                    ps_list[i] = None
                    ptm_list[i] = None

                for i in range(3):
                    scores(i)
                    softmax(i)
                for i in range(3, nch):
                    scores(i)
                    softmax(i)
                    attv(i - 3)
                for i in range(nch - 3, nch):
                    attv(i)

                for h, ps_y in ((ha, ps_ya), (hb, ps_yb)):
                    with nc.allow_low_precision(reason="bf16 y raw"):
                        nc.vector.tensor_copy(
                            out=y_sb[(h % 2) * 64:(h % 2) * 64 + 64,
                                     h // 2, qsl],
                            in_=ps_y[0:64, :])
                    hi = 2 * hp + h % 2
                    with nc.allow_low_precision(reason="bf16 denom"):
                        nc.vector.tensor_copy(out=dtmp[0:1, hi, :],
                                              in_=ps_y[64:65, :])

            # per-qb normalize tail: selector-matmul broadcasts both
            # heads' reciprocal rows into one [128, 512] tile per pair
            den16 = attsm.tile([16, 512], BF16, tag="den16")
            nc.sync.dma_start(out=den16, in_=dtmp[0:1, :, :])
            rden = attsm.tile([16, 512], BF16, tag="rden")
            with nc.allow_low_precision(reason="bf16 softmax denom"):
                nc.vector.reciprocal(out=rden, in_=den16)
            if qb == 1:
                # keep the PE fed while the qb1 denominator DMA +
                # reciprocal complete
                proj_tb(0)

            def norm_tail(qsl=qsl, rden=rden):
                for hp in range(8):
                    rb = pss.tile([128, 2, 512], F32, tag="s")
                    mm(rb[:, 0, :], selt[:, hp, :], rden, True, True)
                    ysl = y_sb[:, hp, qsl]
                    with nc.allow_low_precision(reason="bf16 y norm"):
                        nc.vector.tensor_mul(out=ysl, in0=ysl,
                                             in1=rb[:, 0, :])

            if qb == 0:
                # defer qb0's PE-side tail until qb1's second head
                # pair: the reciprocal chain completes under real work
                pending.append(norm_tail)
            else:
                norm_tail()
        proj_tb(1)

        psy_cm.__exit__(None, None, None)
        pss_cm.__exit__(None, None, None)
        attsm_cm.__exit__(None, None, None)
        attw_cm.__exit__(None, None, None)
        prw_cm.__exit__(None, None, None)
        y_cm.__exit__(None, None, None)
        kv_cm.__exit__(None, None, None)

        # ======== Phase 5: LN2 ========
        ln2x_cm = tc.tile_pool(name="ln2xp", bufs=1)
        ln2xp = ln2x_cm.__enter__()
        ln2x = ln2xp.tile([128, NKC, OWN], BF16)
        w2b_cm = tc.tile_pool(name="ln2wb", bufs=2)
        w2b = w2b_cm.__enter__()
        w2_cm = tc.tile_pool(name="ln2w", bufs=1)
        w2 = w2_cm.__enter__()
        ps2_cm = tc.tile_pool(name="ln2ps", bufs=2, space="PSUM")
        ps2 = ps2_cm.__enter__()
        layer_norm(lambda tb: x2t[:, :, tb * 512:(tb + 1) * 512],
                   ln2x, OWN // 512, ps2, w2, w2b)
        ps2_cm.__exit__(None, None, None)
        w2_cm.__exit__(None, None, None)
        w2b_cm.__exit__(None, None, None)

        # ======== Phase 6: MLP ========
        mlp_cm = tc.tile_pool(name="mlpp", bufs=1)
        mlpp = mlp_cm.__enter__()
        m1t = mlpp.tile([128, NFFC, OWN], BF16)
        mw1_cm = tc.tile_pool(name="mw1", bufs=3)
        mw1 = mw1_cm.__enter__()
        mw2_cm = tc.tile_pool(name="mw2", bufs=2)
        mw2 = mw2_cm.__enter__()
        mo_cm = tc.tile_pool(name="mo", bufs=3)
        mo = mo_cm.__enter__()
        psm_cm = tc.tile_pool(name="mlpps", bufs=4, space="PSUM")
        psm = psm_cm.__enter__()

        # tb-outer: all of block 0's fc1 runs before anything needs
        # ln2x block 1, hiding the LN2 stats tail (wf1 loads twice;
        # the extra 8MB of DMA hides under 55us of matmuls)
        for tb in range(2):
            sl = slice(tb * 512, (tb + 1) * 512)
            for ffc in range(NFFC):
                wt = mw1.tile([128, NKC, 128], BF16, tag="w1")
                nc.sync.dma_start(out=wt, in_=wf1_d[ffc])
                ps = psm.tile([128, 512], F32, tag="mm1")
                for k in range(NKC):
                    mm(ps, wt[:, k, :], ln2x[:, k, sl], k == 0, k == NKC - 1)
                with nc.allow_low_precision(reason="bf16 mlp hidden"):
                    nc.vector.tensor_scalar(
                        out=m1t[:, ffc, sl], in0=ps,
                        scalar1=bf1t[:, ffc:ffc + 1], scalar2=0.0,
                        op0=OP.add, op1=OP.max)
        for oc in range(NKC):
            wt2 = mw2.tile([128, NFFC, 128], BF16, tag="w2")
            nc.sync.dma_start(out=wt2, in_=wf2_d[oc])
            for tb in range(2):
                sl = slice(tb * 512, (tb + 1) * 512)
                ps = psm.tile([128, 512], F32, tag="mm2")
                for k in range(NFFC):
                    mm(ps, wt2[:, k, :], m1t[:, k, sl], k == 0, k == NFFC - 1)
                ot = mo.tile([128, 512], F32, tag="ot")
                nc.vector.scalar_tensor_tensor(
                    out=ot, in0=ps, scalar=bf2t[:, oc:oc + 1],
                    in1=x2t[:, oc, sl], op0=OP.add, op1=OP.add)
                nc.sync.dma_start(out=out_d[:, oc, sl], in_=ot)

        psm_cm.__exit__(None, None, None)
        mo_cm.__exit__(None, None, None)
        mw2_cm.__exit__(None, None, None)
        mw1_cm.__exit__(None, None, None)
        mlp_cm.__exit__(None, None, None)
        ln2x_cm.__exit__(None, None, None)
        x2_cm.__exit__(None, None, None)
        consts_cm.__exit__(None, None, None)

    nc.compile()
    return nc


class _SpmdRunner:
    def __init__(self, nc, n_cores=NC):
        import jax
        from jax.sharding import Mesh, PartitionSpec
        from jax.experimental.shard_map import shard_map
        import concourse.mybir as mybir
        from concourse import bass2jax
        bass2jax.install_neuronx_cc_hook()
        self.jax = jax
        self.n_cores = n_cores
        partition_name = (
            nc.partition_id_tensor.name if nc.partition_id_tensor else None)
        in_names, out_names, out_avals = [], [], []
        for alloc in nc.m.functions[0].allocations:
            if not isinstance(alloc, mybir.MemoryLocationSet):
                continue
            name = alloc.memorylocations[0].name
            if alloc.kind == "ExternalInput":
                if name != partition_name:
                    in_names.append(name)
            elif alloc.kind == "ExternalOutput":
                out_names.append(name)
                out_avals.append(jax.core.ShapedArray(
                    tuple(alloc.tensor_shape), mybir.dt.np(alloc.dtype)))
        self.in_names = in_names
        self.out_names = out_names
        self.out_avals = out_avals
        all_in = in_names + out_names
        if partition_name is not None:
            all_in.append(partition_name)

        def _body(*args):
            operands = list(args)
            if partition_name is not None:
                operands.append(bass2jax.partition_id_tensor())
            outs = bass2jax._bass_exec_p.bind(
                *operands, out_avals=tuple(out_avals),
                in_names=tuple(all_in), out_names=tuple(out_names),
                lowering_input_output_aliases=(),
                sim_require_finite=True, sim_require_nnan=True, nc=nc)
            return tuple(outs)

        devices = jax.devices()[:n_cores]
        self.mesh = Mesh(np.asarray(devices), ("core",))
        n_io = len(in_names) + len(out_names)
        self.fn = jax.jit(
            shard_map(_body, mesh=self.mesh,
                      in_specs=(PartitionSpec("core"),) * n_io,
                      out_specs=(PartitionSpec("core"),) * len(out_names),
                      check_rep=False),
            keep_unused=True)
        self._dev_in = None

    def put_inputs(self, in_maps):
        from jax.sharding import NamedSharding, PartitionSpec
        jax = self.jax
        sh = NamedSharding(self.mesh, PartitionSpec("core"))
        concat = []
        for name in self.in_names:
            arrs = [np.asarray(in_maps[c][name]) for c in range(self.n_cores)]
            concat.append(jax.device_put(np.concatenate(arrs, axis=0), sh))
        for av in self.out_avals:
            z = np.zeros((self.n_cores * av.shape[0], *av.shape[1:]), av.dtype)
            concat.append(jax.device_put(z, sh))
        self._dev_in = concat

    def run(self):
        jax = self.jax
        outs = self.fn(*self._dev_in)
        jax.block_until_ready(outs)
        results = []
        for c in range(self.n_cores):
            d = {}
            for i, name in enumerate(self.out_names):
                av = self.out_avals[i]
                d[name] = np.asarray(outs[i]).reshape(
                    self.n_cores, *av.shape)[c]
            results.append(d)
        return results

    def time_exec(self, warmup=3, m1=4, m2=12, reps=3, trials=6):
        """Estimate per-call device time by dispatching bursts of m1 and
        m2 back-to-back calls and differencing, which cancels the
        constant dispatch/RTT overhead of the axon tunnel. Dispatch
        stalls only ever inflate a burst, so the minimum over several
        trials is the tightest estimate of true device throughput."""
        import time
        jax = self.jax
        for _ in range(warmup):
            jax.block_until_ready(self.fn(*self._dev_in))

        def burst(m):
            t0 = time.perf_counter()
            outs = None
            for _ in range(m):
                outs = self.fn(*self._dev_in)
            jax.block_until_ready(outs)
            return time.perf_counter() - t0

        t1s, t2s = [], []
        for _ in range(trials):
            for _ in range(reps):
                t1s.append(burst(m1))
                t2s.append(burst(m2))
        est = (min(t2s) - min(t1s)) / (m2 - m1)
        if est <= 0:
            # dispatch noise overwhelmed the diff; fall back to the
            # tightest whole-burst bound (includes per-call overhead)
            est = min(min(t2s) / m2, min(t1s) / m1)
        return est


def _get_runner(with_vbias=None):
    if with_vbias is None:
        if "last" in _STATE:
            return _STATE["last"]
        with_vbias = True
    key = ("runner", with_vbias)
    if key not in _STATE:
        nc = _build_program(with_vbias)
        _STATE[key] = _SpmdRunner(nc)
    _STATE["last"] = _STATE[key]
    return _STATE[key]


def _perm(r):
    """Per-core token permutation: own query tokens first (zigzag)."""
    if r == 0:
        return np.concatenate([np.arange(0, 512), np.arange(1536, 2048),
                               np.arange(512, 1536)])
    return np.concatenate([np.arange(512, 1536), np.arange(0, 512),
                           np.arange(1536, 2048)])


def _prep_in_maps(x, W_attn, W_proj, b_proj, W_fc1, b_fc1, W_fc2, b_fc2,
                  ln1_g, ln1_b, ln2_g, ln2_b):
    f32 = np.float32
    bf16 = ml_dtypes.bfloat16
    x = np.asarray(x, f32)
    W_attn = np.asarray(W_attn, f32)
    g1 = np.asarray(ln1_g, f32)
    b1 = np.asarray(ln1_b, f32)
    g2 = np.asarray(ln2_g, f32)
    b2 = np.asarray(ln2_b, f32)
    Wq = g1[:, None] * W_attn[:, 0:C]
    Wk = g1[:, None] * W_attn[:, C:2 * C]
    Wv = g1[:, None] * W_attn[:, 2 * C:3 * C]
    bq = b1 @ W_attn[:, 0:C]
    bk = b1 @ W_attn[:, C:2 * C]
    bv = b1 @ W_attn[:, 2 * C:3 * C]
    Wfc1 = g2[:, None] * np.asarray(W_fc1, f32)
    bfc1 = np.asarray(b_fc1, f32) + b2 @ np.asarray(W_fc1, f32)

    def lhs_tiles(W, nout):
        # [C, nout*128] -> [nout, 128p, NKC, 128m]
        return np.ascontiguousarray(
            W.reshape(NKC, 128, nout, 128).transpose(2, 1, 0, 3)).astype(bf16)

    wq = lhs_tiles(Wq, 8)
    wk = lhs_tiles(Wk, 8)
    wv = np.ascontiguousarray(
        Wv.reshape(NKC, 128, 2, 512).transpose(2, 1, 0, 3)).astype(bf16)
    wp = lhs_tiles(np.asarray(W_proj, f32), 8)
    wf1 = lhs_tiles(Wfc1, NFFC)
    wf2 = np.ascontiguousarray(
        np.asarray(W_fc2, f32).reshape(NFFC, 128, NKC, 128)
        .transpose(2, 1, 0, 3)).astype(bf16)

    def vec(v, nk):
        return np.ascontiguousarray(np.asarray(v, f32).reshape(nk, 128).T)

    sel = np.zeros((16, 8, 128), f32)
    for hp in range(8):
        sel[2 * hp, hp, 0:64] = 1.0
        sel[2 * hp + 1, hp, 64:128] = 1.0
    # LN selectors: ones4 routes block tb's sum to psum row tb; selln
    # broadcasts rnt row tb (rstd, slot 2tb) / row 32+tb (mu*rstd,
    # slot 2tb+1) across all 128 partitions
    ones4 = np.zeros((128, 4, 4), f32)
    for tb in range(4):
        ones4[:, tb, tb] = 1.0
    selln = np.zeros((4, 4, 128), f32)
    for tb in range(4):
        selln[tb, tb, :] = 1.0

    shared = {
        "wq": wq, "wk": wk, "wv": wv, "wp": wp, "wf1": wf1, "wf2": wf2,
        "sel": sel.astype(bf16),
        "ndg": (-60000.0 * np.eye(128, dtype=f32)).astype(bf16),
        "ones4": ones4.astype(bf16),
        "selln": selln.astype(bf16),
        "bq": vec(bq, 8), "bk": vec(bk, 8),
        "bv": np.ascontiguousarray(bv.reshape(1, 2, 512)).astype(bf16),
        "bp": vec(np.asarray(b_proj, f32), NKC),
        "bf1": vec(bfc1, NFFC),
        "bf2": vec(np.asarray(b_fc2, f32), NKC),
    }

    in_maps = []
    for c in range(NC):
        b, r = c // 2, c % 2
        perm = _perm(r)
        xp = x[b][perm]                       # [T, C] local token order
        xt = np.ascontiguousarray(
            xp.T.reshape(NKC, 128, T).transpose(1, 0, 2)).astype(bf16)
        xqf = np.ascontiguousarray(
            xp[:OWN].T.reshape(NKC, 128, OWN).transpose(1, 0, 2))
        # masks[p, slot, qi]: slots 0..7 = qb0 chunks CH_QB0;
        # slots 8..15 = qb1 chunks QB1_MASKED. 1 where kv_g <= q_g.
        # complement masks: 1 where attention is FORBIDDEN (kv > q)
        m = np.zeros((128, 16, 512), f32)
        for j, ci in enumerate(CH_QB0):
            gkv = perm[ci * 128:(ci + 1) * 128]
            gq = perm[0:512]
            m[:, j, :] = (gkv[:, None] > gq[None, :]).astype(f32)
        for j, ci in enumerate(QB1_MASKED):
            gkv = perm[ci * 128:(ci + 1) * 128]
            gq = perm[512:1024]
            m[:, 8 + j, :] = (gkv[:, None] > gq[None, :]).astype(f32)
        d = {"xt": xt, "xqf": xqf, "masks": m.astype(bf16)}
        d.update(shared)
        in_maps.append(d)
    return in_maps


def kernel(x, W_attn, W_proj, b_proj, W_fc1, b_fc1, W_fc2, b_fc2,
           ln1_g, ln1_b, ln2_g, ln2_b):
    bv = np.asarray(ln1_b, np.float32) @ np.asarray(
        W_attn, np.float32)[:, 2 * C:3 * C]
    runner = _get_runner(bool(np.any(bv != 0.0)))
    in_maps = _prep_in_maps(x, W_attn, W_proj, b_proj, W_fc1, b_fc1,
                            W_fc2, b_fc2, ln1_g, ln1_b, ln2_g, ln2_b)
    runner.put_inputs(in_maps)
    results = runner.run()
    out = np.empty((B, T, C), np.float32)
    for c in range(NC):
        b, r = c // 2, c % 2
        ot = results[c]["out"]                # [128, NKC, OWN]
        feat = ot.transpose(1, 0, 2).reshape(C, OWN)
        out[b, _perm(r)[:OWN], :] = feat.T
    return out
